# revision 1
# baseline (speedup 1.0000x reference)
"""Trainium2 Bass kernel for ClassicalSelfAttention.

  out = softmax((X @ R) @ (X @ E).T / sqrt(D)) @ X,  X: (8192, 1024) fp32

Sharding: sequence-parallel over 8 NeuronCores. Core i owns queries
[i*1024, (i+1)*1024). Each core computes its K^T block (E.T @ X_i^T) in two
512-column halves; each half is AllGathered separately so remote key
sub-blocks become usable ~120us earlier than a single monolithic gather.
Attention runs over 16 key sub-blocks of 512 (ring order, chunk-major),
merged in pairs so the online-softmax merge/rescale cost matches the
8-block version.

QK and the projections run in float32r (~13-bit mantissa at full PE rate);
P is cast to bf16 by the exp activation, halving the LDWEIGHTS cost of the
P transposes and the PV weight loads. Accumulation is fp32 in PSUM.
"""
import numpy as np

import concourse.bass as bass_mod
import concourse.bacc as bacc
import concourse.mybir as mybir
from concourse import tile
from concourse.bass_utils import run_bass_kernel_spmd
from concourse.masks import make_identity

DT = mybir.dt
F32 = DT.float32
F32R = DT.float32r
BF16 = DT.bfloat16
ALU = mybir.AluOpType
ACTF = mybir.ActivationFunctionType

S, D, NCORES = 8192, 1024, 8
SL = S // NCORES          # 1024 queries per core
P = 128                   # partitions
DC = D // P               # 8 contraction chunks
MC = SL // P              # 8 query chunks per core
CB = 512                  # key sub-block size (one collective chunk column)
NV = S // CB              # 16 key sub-blocks
TC = CB // P              # 4 t-chunks per sub-block
SCALE = 1.0 / 32.0        # 1/sqrt(D)
NEG_BIG = -1.0e30


def build_program(n_iter=1, bench=None, num_devices=NCORES):
    nc = bacc.Bacc("TRN2", target_bir_lowering=False, debug=False,
                   num_devices=num_devices)

    xt = nc.declare_dram_parameter("xt", [D, SL], F32R, isOutput=False)
    r_p = nc.declare_dram_parameter("r", [D, D], F32R, isOutput=False)
    e_p = nc.declare_dram_parameter("e", [D, D], F32R, isOutput=False)
    x_p = nc.declare_dram_parameter("x", [S, D], F32R, isOutput=False)
    xbf_p = nc.declare_dram_parameter("xbf", [S, D], BF16, isOutput=False)
    out_p = nc.declare_dram_parameter("out", [SL, D], F32, isOutput=True)

    if bench is None:
        bench = n_iter > 1
    import contextlib
    with tile.TileContext(nc) as tc:
        with (
            tc.tile_pool(name="persist", bufs=1) as pers,
            tc.tile_pool(name="dram", bufs=1, space="DRAM") as dram,
            contextlib.ExitStack() as stack,
        ):
            ktb_own = [dram.tile([D, CB], F32R, name=f"ktb_own{c}")
                       for c in range(2)]
            ktb_all = [dram.tile([NCORES * D, CB], F32R,
                                 addr_space="Local" if bench else "Shared",
                                 name=f"ktb_all{c}")
                       for c in range(2)]
            if bench:
                # touch ktb_all once so in-loop reads see written memory
                for c in range(2):
                    nc.sync.dma_start(
                        ktb_all[c][:],
                        x_p[c * NCORES * D:(c + 1) * NCORES * D, 0:CB]
                        .bitcast(F32R))
            if n_iter > 1:
                stack.enter_context(tc.For_i(0, n_iter, 1))

            qt = pers.tile([P, DC * SL], F32R, tag="qt")       # Q^T, [d|m]
            oacc = pers.tile([P, MC * D], F32, tag="oacc")    # O accum per m
            ident32 = pers.tile([P, P], F32, tag="ident32")
            ident_bf = pers.tile([P, P], BF16, tag="identbf")
            # own K^T stays in SBUF for the b=0 pair (no DRAM round trip)
            kst = [pers.tile([P, DC * CB], F32R, tag=f"kst{h}",
                             name=f"kst{h}") for h in range(2)]
            mst = [[pers.tile([P, 1], F32, tag=f"mst{m}_{j}", name=f"mst{m}_{j}")
                    for j in range(2)] for m in range(MC)]
            sig = [pers.tile([P, 1], F32, tag=f"sig{m}", name=f"sig{m}")
                   for m in range(MC)]

            make_identity(nc, ident32[:])
            nc.vector.tensor_copy(ident_bf[:], ident32[:])
            nc.gpsimd.memset(oacc[:], 0.0)
            for m in range(MC):
                nc.gpsimd.memset(mst[m][0][:], NEG_BIG)
                nc.gpsimd.memset(sig[m][:], 0.0)

            # ---------------- Phase A: projections + chunked allgather ----
            with (
                tc.tile_pool(name="pa", bufs=1) as pa,
                tc.tile_pool(name="pa_ps", bufs=2, space="PSUM") as pa_ps,
            ):
                # xt_sb layout: [p, h * (DC*512)]: h-half of the SL columns,
                # then k-chunk of d_in, then 512 cols
                xt_sb = pa.tile([P, DC * SL], F32R, tag="xt")
                e_sb = pa.tile([P, DC * D], F32R, tag="e")    # [d_in | d_out]
                r_sb = pa.tile([P, DC * D], F32R, tag="r")
                # parallel DMA queues (one per trigger engine) so K-proj h0's
                # deps (e + xt-h0) land as fast as possible
                nc.sync.dma_start(
                    e_sb.rearrange("p (k c) -> p k c", k=DC),
                    e_p.rearrange("(k p) c -> p k c", p=P))
                for h, eng in ((0, nc.scalar), (1, nc.gpsimd)):
                    eng.dma_start(
                        xt_sb[:, h * DC * CB:(h + 1) * DC * CB]
                        .rearrange("p (k c) -> p k c", k=DC),
                        xt[:, h * CB:(h + 1) * CB]
                        .rearrange("(k p) c -> p k c", p=P))
                nc.scalar.dma_start(
                    r_sb.rearrange("p (k c) -> p k c", k=DC),
                    r_p.rearrange("(k p) c -> p k c", p=P))

                # K^T own block, h-half at a time: kt = E.T @ X_i^T
                for h in range(2):
                    for o in range(DC):
                        ps = pa_ps.tile([P, CB], F32, tag="proj")
                        for k in range(DC):
                            nc.tensor.matmul(
                                ps[:],
                                e_sb[:, k * D + o * P: k * D + (o + 1) * P],
                                xt_sb[:, h * DC * CB + k * CB:
                                      h * DC * CB + (k + 1) * CB],
                                start=(k == 0), stop=(k == DC - 1),
                            )
                        nc.vector.tensor_copy(
                            kst[h][:, o * CB:(o + 1) * CB], ps[:])
                    nc.sync.dma_start(
                        ktb_own[h].rearrange("(o p) c -> p o c", p=P),
                        kst[h].rearrange("p (o c) -> p o c", o=DC))
                    if bench:
                        # stand-in for the collective with similar traffic
                        nc.gpsimd.dma_start(ktb_all[h][0:D, :], ktb_own[h][:])
                    else:
                        nc.gpsimd.collective_compute(
                            "AllGather",
                            ALU.bypass,
                            replica_groups=[list(range(NCORES))],
                            ins=[ktb_own[h].opt()],
                            outs=[ktb_all[h].opt()],
                        )

                # Q^T: qt = R.T @ X_i^T   [d_out, m]
                for o in range(DC):
                    for h in range(2):
                        ps = pa_ps.tile([P, CB], F32, tag="proj")
                        for k in range(DC):
                            nc.tensor.matmul(
                                ps[:],
                                r_sb[:, k * D + o * P: k * D + (o + 1) * P],
                                xt_sb[:, h * DC * CB + k * CB:
                                      h * DC * CB + (k + 1) * CB],
                                start=(k == 0), stop=(k == DC - 1),
                            )
                        nc.vector.tensor_copy(
                            qt[:, o * SL + h * CB: o * SL + (h + 1) * CB],
                            ps[:])

            # ---------------- Phase B: blocked attention -----------------
            # 16 key sub-blocks of 512 (chunk-major ring order), merged in
            # pairs for the online-softmax update. Software-pipelined: PE
            # runs transposes+PV of a previous pair while DVE/ACT compute
            # stats+exp of the current one.
            with (
                tc.tile_pool(name="kt", bufs=4) as ktp,
                tc.tile_pool(name="xb", bufs=3) as xbp,
                tc.tile_pool(name="ph", bufs=4) as php,
                tc.tile_pool(name="pt", bufs=2) as ptp,
                tc.tile_pool(name="of", bufs=2) as ofp,
                tc.tile_pool(name="stats", bufs=6) as stp,
                tc.tile_pool(name="s_ps", bufs=4, space="PSUM") as sps,
                tc.tile_pool(name="t_ps", bufs=2, space="PSUM") as tps,
                tc.tile_pool(name="o_ps", bufs=1, space="PSUM") as ops,
            ):
                def flush_pe(pend):
                    phs, alpha, m, v, xbs = pend
                    o_part = ops.tile([P, D], F32, tag="opart", name="o_part")
                    pts = []
                    for s in range(2):
                        tp = tps.tile([P, CB], BF16, tag="tp", name="tp")
                        for cc in range(TC):
                            nc.tensor.transpose(
                                tp[:, cc * P:(cc + 1) * P],
                                phs[s][:, cc * P:(cc + 1) * P],
                                ident_bf[:],
                            )
                        pt = ptp.tile([P, CB], BF16, tag="pt", name="pt")
                        nc.scalar.copy(pt[:], tp[:])
                        pts.append(pt)
                    for s in range(2):
                        for cc in range(TC):
                            for h in range(D // CB):
                                nc.tensor.matmul(
                                    o_part[:, h * CB:(h + 1) * CB],
                                    pts[s][:, cc * P:(cc + 1) * P],
                                    xbs[s][:, cc * D + h * CB:
                                           cc * D + (h + 1) * CB],
                                    start=(s == 0 and cc == 0),
                                    stop=(s == 1 and cc == TC - 1),
                                )
                    return o_part

                def flush_dve(pend, o_part):
                    phs, alpha, m, v, xbs = pend
                    nc.vector.scalar_tensor_tensor(
                        oacc[:, m * D:(m + 1) * D],
                        oacc[:, m * D:(m + 1) * D],
                        alpha[:], o_part[:],
                        op0=ALU.mult, op1=ALU.add)
                    if v == NV // 2 - 1:
                        # finalize this m: divide by softmax sum and store
                        rcp = stp.tile([P, 1], F32, tag="rcp", name="rcp")
                        nc.vector.reciprocal(rcp[:], sig[m][:])
                        of = ofp.tile([P, D], F32, tag="ofin", name="ofin")
                        nc.vector.tensor_scalar_mul(
                            of[:], oacc[:, m * D:(m + 1) * D], rcp[:])
                        nc.sync.dma_start(out_p[m * P:(m + 1) * P, :], of[:])

                pending = []
                pid = nc.sync.partition_id()
                pid_a = nc.scalar.partition_id()
                pid_g = nc.gpsimd.partition_id()
                # sub-block visit order: own block's two halves first (no
                # collective dependency), then chunk-major ring order so
                # chunk-0 sub-blocks are consumed while chunk 1 gathers.
                visits = [(0, 0), (0, 1)]
                for c in range(2):
                    visits += [(b, c) for b in range(1, NCORES)]
                for v in range(NV // 2):
                    sub = [visits[2 * v], visits[2 * v + 1]]
                    kts, xbs = [], []
                    for si, (b, c) in enumerate(sub):
                        if b == 0:
                            # own K^T already sits in SBUF from phase A
                            kts.append(kst[c])
                        else:
                            kt = ktp.tile([P, DC * CB], F32R, tag="kt",
                                          name="kt")
                            kpid = pid if si == 0 else pid_a
                            (nc.sync if si == 0 else nc.scalar).dma_start(
                                kt.rearrange("p (k c) -> p k c", k=DC),
                                ktb_all[c][
                                    bass_mod.ds(((kpid + b) % NCORES) * D, D), :]
                                .rearrange("(k p) c -> p k c", p=P))
                            kts.append(kt)
                        # PV runs in bf16 (matmul can't mix f32r/bf16); X is
                        # pre-cast to bf16 on the host
                        xb = xbp.tile([P, TC * D], BF16, tag="xb", name="xb")
                        nc.gpsimd.dma_start(
                            xb.rearrange("p (k c) -> p k c", k=TC),
                            xbf_p[bass_mod.ds(
                                ((pid_g + b) % NCORES) * SL + c * CB, CB), :]
                            .rearrange("(k p) c -> p k c", p=P))
                        xbs.append(xb)

                    for m in range(MC):
                        sh_ = [sps.tile([P, CB], F32, tag="s", name="s")
                               for _ in range(2)]
                        mqh = [stp.tile([P, 1], F32, tag=f"mq{s}",
                                        name=f"mq{s}") for s in range(2)]
                        for s in range(2):
                            for k in range(DC):
                                nc.tensor.matmul(
                                    sh_[s][:],
                                    qt[:, k * SL + m * P: k * SL + (m + 1) * P],
                                    kts[s][:, k * CB:(k + 1) * CB],
                                    start=(k == 0), stop=(k == DC - 1),
                                )
                            nc.vector.reduce_max(mqh[s][:], sh_[s][:],
                                                 axis=mybir.AxisListType.X)

                        # online softmax stats; mst ping-pongs on v parity
                        m_old = mst[m][v % 2]
                        mnew = mst[m][(v + 1) % 2]
                        mq = stp.tile([P, 1], F32, tag="mq", name="mq")
                        nc.vector.tensor_max(mq[:], mqh[0][:], mqh[1][:])
                        nc.vector.tensor_max(mnew[:], m_old[:], mq[:])
                        nbias = stp.tile([P, 1], F32, tag="nbias", name="nbias")
                        nc.scalar.mul(nbias[:], mnew[:], -SCALE)
                        # alpha = exp((m_old - mnew)/32)
                        alpha = stp.tile([P, 1], F32, tag="alpha", name="alpha")
                        nc.scalar.activation(alpha[:], m_old[:], ACTF.Exp,
                                             bias=nbias[:], scale=SCALE)

                        # phat = exp(s/32 - mnew/32) in bf16; sums into sq
                        phs = []
                        sqh = [stp.tile([P, 1], F32, tag=f"sq{s}",
                                        name=f"sq{s}") for s in range(2)]
                        for s in range(2):
                            ph = php.tile([P, CB], BF16, tag="ph", name="ph")
                            nc.scalar.activation(ph[:], sh_[s][:], ACTF.Exp,
                                                 bias=nbias[:], scale=SCALE,
                                                 accum_out=sqh[s][:])
                            phs.append(ph)
                        sq = stp.tile([P, 1], F32, tag="sq", name="sq")
                        nc.vector.tensor_add(sq[:], sqh[0][:], sqh[1][:])
                        nc.vector.scalar_tensor_tensor(
                            sig[m][:], sig[m][:], alpha[:], sq[:],
                            op0=ALU.mult, op1=ALU.add)

                        pending.append((phs, alpha, m, v, xbs))
                        if len(pending) > 2:
                            pend_fl = pending.pop(0)
                            flush_dve(pend_fl, flush_pe(pend_fl))
                for pend in pending:
                    flush_dve(pend, flush_pe(pend))

    nc.compile()
    return nc


_PROGRAM = None


def _get_program():
    global _PROGRAM
    if _PROGRAM is None:
        _PROGRAM = build_program()
    return _PROGRAM


def kernel(inputs, rotation_params, entangle_params, _trace=False):
    X = np.ascontiguousarray(np.asarray(inputs, dtype=np.float32))
    R = np.ascontiguousarray(np.asarray(rotation_params, dtype=np.float32))
    E = np.ascontiguousarray(np.asarray(entangle_params, dtype=np.float32))
    assert X.shape == (S, D) and R.shape == (D, D) and E.shape == (D, D)

    import ml_dtypes
    XT = np.ascontiguousarray(X.T)
    Xbf = np.ascontiguousarray(X.astype(ml_dtypes.bfloat16))
    in_maps = []
    for i in range(NCORES):
        in_maps.append({
            "xt": np.ascontiguousarray(XT[:, i * SL:(i + 1) * SL]),
            "r": R,
            "e": E,
            "x": X,
            "xbf": Xbf,
        })

    nc = _get_program()
    res = run_bass_kernel_spmd(nc, in_maps, list(range(NCORES)),
                               trace=_trace)
    out = np.concatenate([res.results[i]["out"] for i in range(NCORES)],
                         axis=0)
    if _trace:
        return out, res
    return out



# revision 2
# speedup vs baseline: 1.1557x; 1.1557x over previous
"""Trainium2 Bass kernel for ClassicalSelfAttention.

  out = softmax((X @ R) @ (X @ E).T / sqrt(D)) @ X,  X: (8192, 1024) fp32

Key identity: scores = (X R)(X E)^T = X (R E^T) X^T.  Each core computes
W = R @ E^T redundantly (27us of PE), projects its own query slice
T^T = W^T-free form (lhsT=W, rhs=X^T_own), and then the "keys" are just
X^T itself — which every core already holds in DRAM.  No collectives at
all: the attention loop streams X^T / X blocks straight from HBM.

Sequence-parallel over 8 NeuronCores: core i owns queries
[i*1024, (i+1)*1024).  Attention runs over 16 key blocks of 512 with a
standard online softmax (per-query running max on DVE, exp on ACT,
P^T via PE transposes, PV accumulated in PSUM then merged into SBUF).

QK and the projections run in float32r (~13-bit mantissa at full PE
rate) — bf16 logits would flip argmaxes of this extremely peaked
softmax.  P is cast to bf16 by the exp activation; PV runs bf16.
"""
import numpy as np

import concourse.bass as bass_mod
import concourse.bacc as bacc
import concourse.mybir as mybir
from concourse import tile
from concourse.bass_utils import run_bass_kernel_spmd
from concourse.masks import make_identity

DT = mybir.dt
F32 = DT.float32
F32R = DT.float32r
BF16 = DT.bfloat16
ALU = mybir.AluOpType
ACTF = mybir.ActivationFunctionType

S, D, NCORES = 8192, 1024, 8
SL = S // NCORES          # 1024 queries per core
P = 128                   # partitions
DC = D // P               # 8 contraction chunks
MC = SL // P              # 8 query chunks per core
CB = 512                  # key block size
NV = S // CB              # 16 key blocks
TC = CB // P              # 4 t-chunks per block
SCALE = 1.0 / 32.0        # 1/sqrt(D)
NEG_BIG = -1.0e30


def build_program(num_devices=NCORES):
    nc = bacc.Bacc("TRN2", target_bir_lowering=False, debug=False,
                   num_devices=num_devices)

    rt_p = nc.declare_dram_parameter("rt", [D, D], F32R, isOutput=False)
    et_p = nc.declare_dram_parameter("et", [D, D], F32R, isOutput=False)
    xt_p = nc.declare_dram_parameter("xt", [D, SL], F32R, isOutput=False)
    xtf_p = nc.declare_dram_parameter("xtf", [D, S], F32R, isOutput=False)
    xbf_p = nc.declare_dram_parameter("xbf", [S, D], BF16, isOutput=False)
    out_p = nc.declare_dram_parameter("out", [SL, D], F32, isOutput=True)

    with tile.TileContext(nc) as tc:
        with (
            tc.tile_pool(name="persist", bufs=1) as pers,
        ):
            tq = pers.tile([P, DC * SL], F32R, tag="tq")      # T^T, [c|m]
            oacc = pers.tile([P, MC * D], F32, tag="oacc")    # O accum per m
            ident_bf = pers.tile([P, P], BF16, tag="identbf")
            mst = [[pers.tile([P, 1], F32, tag=f"mst{m}_{j}",
                              name=f"mst{m}_{j}")
                    for j in range(2)] for m in range(MC)]
            sig = [pers.tile([P, 1], F32, tag=f"sig{m}", name=f"sig{m}")
                   for m in range(MC)]

            ident32 = None
            with tc.tile_pool(name="ident_tmp", bufs=1) as identp:
                ident32 = identp.tile([P, P], F32, tag="ident32")
                make_identity(nc, ident32[:])
                nc.vector.tensor_copy(ident_bf[:], ident32[:])
            nc.gpsimd.memset(oacc[:], 0.0)
            for m in range(MC):
                nc.gpsimd.memset(mst[m][0][:], NEG_BIG)
                nc.gpsimd.memset(sig[m][:], 0.0)

            # ---------------- Phase A: W = R @ E^T, then T^T ----------------
            with (
                tc.tile_pool(name="pa", bufs=1) as pa,
                tc.tile_pool(name="pa_ps", bufs=2, space="PSUM") as pa_ps,
            ):
                rt_sb = pa.tile([P, DC * D], F32R, tag="rt")  # [d_in | i]
                et_sb = pa.tile([P, DC * D], F32R, tag="et")  # [d_in | j]
                xt_sb = pa.tile([P, DC * SL], F32R, tag="xt")  # [d_in | m]
                w_sb = pa.tile([P, DC * D], F32R, tag="w")    # [i | j]
                nc.sync.dma_start(
                    rt_sb.rearrange("p (k c) -> p k c", k=DC),
                    rt_p.rearrange("(k p) c -> p k c", p=P))
                nc.scalar.dma_start(
                    et_sb.rearrange("p (k c) -> p k c", k=DC),
                    et_p.rearrange("(k p) c -> p k c", p=P))
                nc.gpsimd.dma_start(
                    xt_sb.rearrange("p (k c) -> p k c", k=DC),
                    xt_p.rearrange("(k p) c -> p k c", p=P))

                # W[i, j] = sum_d R^T[d, i] * E^T[d, j]
                for i in range(DC):
                    for jh in range(2):
                        ps = pa_ps.tile([P, CB], F32, tag="proj")
                        for k in range(DC):
                            nc.tensor.matmul(
                                ps[:],
                                rt_sb[:, k * D + i * P: k * D + (i + 1) * P],
                                et_sb[:, k * D + jh * CB:
                                      k * D + (jh + 1) * CB],
                                start=(k == 0), stop=(k == DC - 1),
                            )
                        nc.vector.tensor_copy(
                            w_sb[:, i * D + jh * CB: i * D + (jh + 1) * CB],
                            ps[:])

                # T^T[c, m] = sum_d W[d, c] * X^T[d, m]
                for c in range(DC):
                    for mh in range(2):
                        ps = pa_ps.tile([P, CB], F32, tag="proj")
                        for k in range(DC):
                            nc.tensor.matmul(
                                ps[:],
                                w_sb[:, k * D + c * P: k * D + (c + 1) * P],
                                xt_sb[:, k * SL + mh * CB:
                                      k * SL + (mh + 1) * CB],
                                start=(k == 0), stop=(k == DC - 1),
                            )
                        nc.vector.tensor_copy(
                            tq[:, c * SL + mh * CB: c * SL + (mh + 1) * CB],
                            ps[:])

            # ---------------- Phase B: blocked attention -----------------
            # 16 key blocks of 512.  Software-pipelined: PE runs
            # transposes+PV of a previous block's m while DVE/ACT compute
            # stats+exp of the current one.
            with (
                tc.tile_pool(name="kt", bufs=3) as ktp,
                tc.tile_pool(name="xb", bufs=3) as xbp,
                tc.tile_pool(name="ph", bufs=4) as php,
                tc.tile_pool(name="pt", bufs=2) as ptp,
                tc.tile_pool(name="of", bufs=2) as ofp,
                tc.tile_pool(name="stats", bufs=6) as stp,
                tc.tile_pool(name="s_ps", bufs=3, space="PSUM") as sps,
                tc.tile_pool(name="t_ps", bufs=2, space="PSUM") as tps,
                tc.tile_pool(name="o_ps", bufs=1, space="PSUM") as ops,
            ):
                def flush_pe(pend):
                    ph, alpha, m, b, xb = pend
                    tp = tps.tile([P, CB], BF16, tag="tp", name="tp")
                    for tc_ in range(TC):
                        nc.tensor.transpose(
                            tp[:, tc_ * P:(tc_ + 1) * P],
                            ph[:, tc_ * P:(tc_ + 1) * P],
                            ident_bf[:],
                        )
                    pt = ptp.tile([P, CB], BF16, tag="pt", name="pt")
                    nc.scalar.copy(pt[:], tp[:])
                    o_part = ops.tile([P, D], F32, tag="opart", name="o_part")
                    for tc_ in range(TC):
                        for h in range(D // CB):
                            nc.tensor.matmul(
                                o_part[:, h * CB:(h + 1) * CB],
                                pt[:, tc_ * P:(tc_ + 1) * P],
                                xb[:, tc_ * D + h * CB:
                                   tc_ * D + (h + 1) * CB],
                                start=(tc_ == 0), stop=(tc_ == TC - 1),
                            )
                    return o_part

                def flush_dve(pend, o_part):
                    ph, alpha, m, b, xb = pend
                    nc.vector.scalar_tensor_tensor(
                        oacc[:, m * D:(m + 1) * D],
                        oacc[:, m * D:(m + 1) * D],
                        alpha[:], o_part[:],
                        op0=ALU.mult, op1=ALU.add)
                    if b == NV - 1:
                        # finalize this m: divide by softmax sum and store
                        rcp = stp.tile([P, 1], F32, tag="rcp", name="rcp")
                        nc.vector.reciprocal(rcp[:], sig[m][:])
                        of = ofp.tile([P, D], F32, tag="ofin", name="ofin")
                        nc.vector.tensor_scalar_mul(
                            of[:], oacc[:, m * D:(m + 1) * D], rcp[:])
                        nc.sync.dma_start(out_p[m * P:(m + 1) * P, :], of[:])

                pending = []
                for b in range(NV):
                    kt = ktp.tile([P, DC * CB], F32R, tag="kt", name="kt")
                    (nc.sync if b % 2 == 0 else nc.scalar).dma_start(
                        kt.rearrange("p (k c) -> p k c", k=DC),
                        xtf_p[:, b * CB:(b + 1) * CB]
                        .rearrange("(k p) c -> p k c", p=P))
                    xb = xbp.tile([P, TC * D], BF16, tag="xb", name="xb")
                    nc.gpsimd.dma_start(
                        xb.rearrange("p (k c) -> p k c", k=TC),
                        xbf_p[b * CB:(b + 1) * CB, :]
                        .rearrange("(k p) c -> p k c", p=P))

                    for m in range(MC):
                        sh = sps.tile([P, CB], F32, tag="s", name="s")
                        for k in range(DC):
                            nc.tensor.matmul(
                                sh[:],
                                tq[:, k * SL + m * P: k * SL + (m + 1) * P],
                                kt[:, k * CB:(k + 1) * CB],
                                start=(k == 0), stop=(k == DC - 1),
                            )
                        mq = stp.tile([P, 1], F32, tag="mq", name="mq")
                        nc.vector.reduce_max(mq[:], sh[:],
                                             axis=mybir.AxisListType.X)

                        # online softmax stats; mst ping-pongs on b parity
                        m_old = mst[m][b % 2]
                        mnew = mst[m][(b + 1) % 2]
                        nc.vector.tensor_max(mnew[:], m_old[:], mq[:])
                        nbias = stp.tile([P, 1], F32, tag="nbias",
                                         name="nbias")
                        nc.scalar.mul(nbias[:], mnew[:], -SCALE)
                        # alpha = exp((m_old - mnew)/32)
                        alpha = stp.tile([P, 1], F32, tag="alpha",
                                         name="alpha")
                        nc.scalar.activation(alpha[:], m_old[:], ACTF.Exp,
                                             bias=nbias[:], scale=SCALE)

                        # phat = exp(s/32 - mnew/32) in bf16; row sums in sq
                        sq = stp.tile([P, 1], F32, tag="sq", name="sq")
                        ph = php.tile([P, CB], BF16, tag="ph", name="ph")
                        nc.scalar.activation(ph[:], sh[:], ACTF.Exp,
                                             bias=nbias[:], scale=SCALE,
                                             accum_out=sq[:])
                        nc.vector.scalar_tensor_tensor(
                            sig[m][:], sig[m][:], alpha[:], sq[:],
                            op0=ALU.mult, op1=ALU.add)

                        pending.append((ph, alpha, m, b, xb))
                        if len(pending) > 2:
                            pend_fl = pending.pop(0)
                            flush_dve(pend_fl, flush_pe(pend_fl))
                for pend in pending:
                    flush_dve(pend, flush_pe(pend))

    nc.compile()
    return nc


_PROGRAM = None


def _get_program():
    global _PROGRAM
    if _PROGRAM is None:
        _PROGRAM = build_program()
    return _PROGRAM


def kernel(inputs, rotation_params, entangle_params, _trace=False):
    X = np.ascontiguousarray(np.asarray(inputs, dtype=np.float32))
    R = np.ascontiguousarray(np.asarray(rotation_params, dtype=np.float32))
    E = np.ascontiguousarray(np.asarray(entangle_params, dtype=np.float32))
    assert X.shape == (S, D) and R.shape == (D, D) and E.shape == (D, D)

    import ml_dtypes
    XT = np.ascontiguousarray(X.T)
    RT = np.ascontiguousarray(R.T)
    ET = np.ascontiguousarray(E.T)
    Xbf = np.ascontiguousarray(X.astype(ml_dtypes.bfloat16))
    in_maps = []
    for i in range(NCORES):
        in_maps.append({
            "rt": RT,
            "et": ET,
            "xt": np.ascontiguousarray(XT[:, i * SL:(i + 1) * SL]),
            "xtf": XT,
            "xbf": Xbf,
        })

    nc = _get_program()
    res = run_bass_kernel_spmd(nc, in_maps, list(range(NCORES)),
                               trace=_trace)
    out = np.concatenate([res.results[i]["out"] for i in range(NCORES)],
                         axis=0)
    if _trace:
        return out, res
    return out


# revision 9
# speedup vs baseline: 1.2050x; 1.0427x over previous
"""Trainium2 Bass kernel for ClassicalSelfAttention.

  out = softmax((X @ R) @ (X @ E).T / sqrt(D)) @ X,  X: (8192, 1024) fp32

Key identity: scores = (X R)(X E)^T = X (R E^T) X^T.  Each core computes
W = R @ E^T redundantly (27us of PE), projects its own query slice
(T^T with lhsT=W, rhs=X^T_own), and then the "keys" are just X^T itself
— which every core already holds in DRAM.  No collectives at all: the
attention loop streams X^T / X blocks straight from HBM.

Sequence-parallel over 8 NeuronCores: core i owns queries
[i*1024, (i+1)*1024).  Attention runs over 16 key blocks of 512 with a
standard online softmax (per-query running max on DVE, exp on ACT, PV
accumulated in PSUM then merged into SBUF).  P^T for the PV matmul is
produced by PE transposes against a bf16 identity.

QK and the projections run in float32r (~13-bit mantissa at full PE
rate) — bf16 logits would flip argmaxes of this extremely peaked
softmax.  P is cast to bf16 by the exp activation; PV runs bf16.

Startup DMAs are split by first-consumer order (rt in 128-col blocks,
et/xt in halves) across the two HWDGE queues so the first W matmul
issues at ~12us instead of waiting for whole-tensor loads.
"""
import numpy as np

import concourse.bass as bass_mod
import concourse.bacc as bacc
import concourse.mybir as mybir
from concourse import tile
from concourse.bass_utils import run_bass_kernel_spmd
from concourse.masks import make_identity

DT = mybir.dt
F32 = DT.float32
F32R = DT.float32r
BF16 = DT.bfloat16
ALU = mybir.AluOpType
ACTF = mybir.ActivationFunctionType

S, D, NCORES = 8192, 1024, 8
SL = S // NCORES          # 1024 queries per core
P = 128                   # partitions
DC = D // P               # 8 contraction chunks
MC = SL // P              # 8 query chunks per core
CB = 512                  # key block size
NV = S // CB              # 16 key blocks
TC = CB // P              # 4 t-chunks per block
SCALE = 1.0 / 32.0        # 1/sqrt(D)
NEG_BIG = -1.0e30


def build_program(num_devices=NCORES):
    nc = bacc.Bacc("TRN2", target_bir_lowering=False, debug=False,
                   num_devices=num_devices)

    rt_p = nc.declare_dram_parameter("rt", [D, D], F32R, isOutput=False)
    et_p = nc.declare_dram_parameter("et", [D, D], F32R, isOutput=False)
    xt_p = nc.declare_dram_parameter("xt", [D, SL], F32R, isOutput=False)
    xtf_p = nc.declare_dram_parameter("xtf", [D, S], F32R, isOutput=False)
    xbf_p = nc.declare_dram_parameter("xbf", [S, D], BF16, isOutput=False)
    out_p = nc.declare_dram_parameter("out", [SL, D], F32, isOutput=True)

    with tile.TileContext(nc) as tc:
        with (
            tc.tile_pool(name="persist", bufs=1) as pers,
        ):
            tq = pers.tile([P, DC * SL], F32R, tag="tq")      # T^T, [c|m]
            oacc = pers.tile([P, MC * D], F32, tag="oacc")    # O accum per m
            ident_bf = pers.tile([P, P], BF16, tag="identbf")
            mst = [[pers.tile([P, 1], F32, tag=f"mst{m}_{j}",
                              name=f"mst{m}_{j}")
                    for j in range(2)] for m in range(MC)]
            sig = [pers.tile([P, 1], F32, tag=f"sig{m}", name=f"sig{m}")
                   for m in range(MC)]

            with tc.tile_pool(name="ident_tmp", bufs=1) as identp:
                ident32 = identp.tile([P, P], F32, tag="ident32")
                make_identity(nc, ident32[:])
                nc.vector.tensor_copy(ident_bf[:], ident32[:])
            nc.vector.memset(oacc[:], 0.0)
            for m in range(MC):
                nc.vector.memset(mst[m][0][:], NEG_BIG)
                nc.vector.memset(sig[m][:], 0.0)

            # ---------------- Phase A: W = R @ E^T, then T^T ----------------
            with (
                tc.tile_pool(name="pa", bufs=1) as pa,
                tc.tile_pool(name="pa_ps", bufs=2, space="PSUM") as pa_ps,
            ):
                rt_sb = pa.tile([P, DC * D], F32R, tag="rt")  # [d_in | i]
                et_sb = pa.tile([P, DC * D], F32R, tag="et")  # [d_in | j]
                xt_sb = pa.tile([P, DC * SL], F32R, tag="xt")  # [d_in | m]
                w_sb = pa.tile([P, DC * D], F32R, tag="w")    # [i | j]
                rt_r = rt_sb.rearrange("p (k c) -> p k c", k=DC)
                rt_src = rt_p.rearrange("(k p) c -> p k c", p=P)
                # rt split by i-block (the W stationary slices), first first
                for i in range(DC):
                    nc.sync.dma_start(rt_r[:, :, i * P:(i + 1) * P],
                                      rt_src[:, :, i * P:(i + 1) * P])
                et_r = et_sb.rearrange("p (k c) -> p k c", k=DC)
                et_src = et_p.rearrange("(k p) c -> p k c", p=P)
                for jh in range(2):
                    nc.scalar.dma_start(et_r[:, :, jh * CB:(jh + 1) * CB],
                                        et_src[:, :, jh * CB:(jh + 1) * CB])
                xt_r = xt_sb.rearrange("p (k c) -> p k c", k=DC)
                xt_src = xt_p.rearrange("(k p) c -> p k c", p=P)
                nc.sync.dma_start(xt_r[:, :, 0:CB], xt_src[:, :, 0:CB])
                nc.scalar.dma_start(xt_r[:, :, CB:2 * CB],
                                    xt_src[:, :, CB:2 * CB])

                # W[i, j] = sum_d R^T[d, i] * E^T[d, j]
                for jh in range(2):
                    for i in range(DC):
                        ps = pa_ps.tile([P, CB], F32, tag="proj")
                        for k in range(DC):
                            nc.tensor.matmul(
                                ps[:],
                                rt_sb[:, k * D + i * P: k * D + (i + 1) * P],
                                et_sb[:, k * D + jh * CB:
                                      k * D + (jh + 1) * CB],
                                start=(k == 0), stop=(k == DC - 1),
                            )
                        nc.vector.tensor_copy(
                            w_sb[:, i * D + jh * CB: i * D + (jh + 1) * CB],
                            ps[:])

                # T^T[c, m] = sum_d W[d, c] * X^T[d, m]
                for mh in range(2):
                    for c in range(DC):
                        ps = pa_ps.tile([P, CB], F32, tag="proj")
                        for k in range(DC):
                            nc.tensor.matmul(
                                ps[:],
                                w_sb[:, k * D + c * P: k * D + (c + 1) * P],
                                xt_sb[:, k * SL + mh * CB:
                                      k * SL + (mh + 1) * CB],
                                start=(k == 0), stop=(k == DC - 1),
                            )
                        nc.vector.tensor_copy(
                            tq[:, c * SL + mh * CB: c * SL + (mh + 1) * CB],
                            ps[:])

            # ---------------- Phase B: blocked attention -----------------
            # 16 key blocks of 512.  Software-pipelined: PE runs PV of a
            # previous block's m while DVE/ACT compute stats+exp of the
            # current one and DMA xbar transposes P.
            with (
                tc.tile_pool(name="kt", bufs=3) as ktp,
                tc.tile_pool(name="xb", bufs=3) as xbp,
                tc.tile_pool(name="ph", bufs=4) as php,
                tc.tile_pool(name="pt", bufs=3) as ptp,
                tc.tile_pool(name="of", bufs=2) as ofp,
                tc.tile_pool(name="stats", bufs=6) as stp,
                tc.tile_pool(name="s_ps", bufs=4, space="PSUM") as sps,
                tc.tile_pool(name="t_ps", bufs=2, space="PSUM") as tps,
                tc.tile_pool(name="o_ps", bufs=1, space="PSUM") as ops,
            ):
                def flush_pe(pend):
                    ph, alpha, m, b, xb = pend
                    tp = tps.tile([P, CB], BF16, tag="tp", name="tp")
                    for tc_ in range(TC):
                        nc.tensor.transpose(
                            tp[:, tc_ * P:(tc_ + 1) * P],
                            ph[:, tc_ * P:(tc_ + 1) * P],
                            ident_bf[:],
                        )
                    pt = ptp.tile([P, CB], BF16, tag="pt", name="pt")
                    nc.scalar.copy(pt[:], tp[:])
                    o_part = ops.tile([P, D], F32, tag="opart", name="o_part")
                    for tc_ in range(TC):
                        for h in range(D // CB):
                            nc.tensor.matmul(
                                o_part[:, h * CB:(h + 1) * CB],
                                pt[:, tc_ * P:(tc_ + 1) * P],
                                xb[:, tc_ * D + h * CB:
                                   tc_ * D + (h + 1) * CB],
                                start=(tc_ == 0), stop=(tc_ == TC - 1),
                            )
                    return o_part

                def flush_dve(pend, o_part):
                    ph, alpha, m, b, xb = pend
                    nc.vector.scalar_tensor_tensor(
                        oacc[:, m * D:(m + 1) * D],
                        oacc[:, m * D:(m + 1) * D],
                        alpha[:], o_part[:],
                        op0=ALU.mult, op1=ALU.add)
                    if b == NV - 1:
                        # finalize this m: divide by softmax sum and store
                        rcp = stp.tile([P, 1], F32, tag="rcp", name="rcp")
                        nc.vector.reciprocal(rcp[:], sig[m][:])
                        of = ofp.tile([P, D], F32, tag="ofin", name="ofin")
                        nc.vector.tensor_scalar_mul(
                            of[:], oacc[:, m * D:(m + 1) * D], rcp[:])
                        nc.sync.dma_start(out_p[m * P:(m + 1) * P, :], of[:])

                pending = []
                for b in range(NV):
                    kt = ktp.tile([P, DC * CB], F32R, tag="kt", name="kt")
                    nc.sync.dma_start(
                        kt.rearrange("p (k c) -> p k c", k=DC),
                        xtf_p[:, b * CB:(b + 1) * CB]
                        .rearrange("(k p) c -> p k c", p=P))
                    xb = xbp.tile([P, TC * D], BF16, tag="xb", name="xb")
                    nc.gpsimd.dma_start(
                        xb.rearrange("p (k c) -> p k c", k=TC),
                        xbf_p[b * CB:(b + 1) * CB, :]
                        .rearrange("(k p) c -> p k c", p=P))

                    for m in range(MC):
                        sh = sps.tile([P, CB], F32, tag="s", name="s")
                        for k in range(DC):
                            nc.tensor.matmul(
                                sh[:],
                                tq[:, k * SL + m * P: k * SL + (m + 1) * P],
                                kt[:, k * CB:(k + 1) * CB],
                                start=(k == 0), stop=(k == DC - 1),
                            )
                        mq = stp.tile([P, 1], F32, tag="mq", name="mq")
                        nc.vector.reduce_max(mq[:], sh[:],
                                             axis=mybir.AxisListType.X)

                        # online softmax stats; mst ping-pongs on b parity
                        m_old = mst[m][b % 2]
                        mnew = mst[m][(b + 1) % 2]
                        nc.vector.tensor_max(mnew[:], m_old[:], mq[:])
                        nbias = stp.tile([P, 1], F32, tag="nbias",
                                         name="nbias")
                        nc.scalar.mul(nbias[:], mnew[:], -SCALE)
                        # alpha = exp((m_old - mnew)/32)
                        alpha = stp.tile([P, 1], F32, tag="alpha",
                                         name="alpha")
                        nc.scalar.activation(alpha[:], m_old[:], ACTF.Exp,
                                             bias=nbias[:], scale=SCALE)

                        # phat = exp(s/32 - mnew/32) in bf16; row sums in sq
                        sq = stp.tile([P, 1], F32, tag="sq", name="sq")
                        ph = php.tile([P, CB], BF16, tag="ph", name="ph")
                        nc.scalar.activation(ph[:], sh[:], ACTF.Exp,
                                             bias=nbias[:], scale=SCALE,
                                             accum_out=sq[:])
                        nc.vector.scalar_tensor_tensor(
                            sig[m][:], sig[m][:], alpha[:], sq[:],
                            op0=ALU.mult, op1=ALU.add)

                        pending.append((ph, alpha, m, b, xb))
                        if len(pending) > 2:
                            pend_fl = pending.pop(0)
                            flush_dve(pend_fl, flush_pe(pend_fl))
                for pend in pending:
                    flush_dve(pend, flush_pe(pend))

    nc.compile()
    return nc


_PROGRAM = None


def _get_program():
    global _PROGRAM
    if _PROGRAM is None:
        _PROGRAM = build_program()
    return _PROGRAM


def kernel(inputs, rotation_params, entangle_params, _trace=False):
    X = np.ascontiguousarray(np.asarray(inputs, dtype=np.float32))
    R = np.ascontiguousarray(np.asarray(rotation_params, dtype=np.float32))
    E = np.ascontiguousarray(np.asarray(entangle_params, dtype=np.float32))
    assert X.shape == (S, D) and R.shape == (D, D) and E.shape == (D, D)

    import ml_dtypes
    XT = np.ascontiguousarray(X.T)
    RT = np.ascontiguousarray(R.T)
    ET = np.ascontiguousarray(E.T)
    Xbf = np.ascontiguousarray(X.astype(ml_dtypes.bfloat16))
    in_maps = []
    for i in range(NCORES):
        in_maps.append({
            "rt": RT,
            "et": ET,
            "xt": np.ascontiguousarray(XT[:, i * SL:(i + 1) * SL]),
            "xtf": XT,
            "xbf": Xbf,
        })

    nc = _get_program()
    res = run_bass_kernel_spmd(nc, in_maps, list(range(NCORES)),
                               trace=_trace)
    out = np.concatenate([res.results[i]["out"] for i in range(NCORES)],
                         axis=0)
    if _trace:
        return out, res
    return out


# revision 15
# speedup vs baseline: 1.2317x; 1.0221x over previous
"""Trainium2 Bass kernel for ClassicalSelfAttention.

  out = softmax((X @ R) @ (X @ E).T / sqrt(D)) @ X,  X: (8192, 1024) fp32

Key identity: scores = (X R)(X E)^T = X (R E^T) X^T.  Each core computes
W = R @ E^T redundantly (27us of PE), projects its own query slice
(T^T with lhsT=W, rhs=X^T_own), and then the "keys" are just X^T itself
— which every core already holds in DRAM.  No collectives at all: the
attention loop streams X^T / X blocks straight from HBM.

Sequence-parallel over 8 NeuronCores: core i owns queries
[i*1024, (i+1)*1024).  Attention runs over 16 key blocks of 512, ring
order starting at the core's own two blocks (which double as the T^T
moving operand — no separate X^T_own load), with a standard online
softmax (per-query running max on DVE, exp on ACT, PV accumulated in
PSUM then merged into SBUF).  P^T for the PV matmul is produced by PE
transposes against a bf16 identity.

QK and the projections run in float32r (~13-bit mantissa at full PE
rate) — bf16 logits would flip argmaxes of this extremely peaked
softmax.  P is cast to bf16 by the exp activation; PV runs bf16.

Startup DMAs are split by first-consumer order (rt in i-pair slabs, et
in halves) across the two HWDGE queues so the first W matmul issues
early instead of waiting for whole-tensor loads.
"""
import numpy as np

import concourse.bass as bass_mod
import concourse.bacc as bacc
import concourse.mybir as mybir
from concourse import tile
from concourse.bass_utils import run_bass_kernel_spmd
from concourse.masks import make_identity

DT = mybir.dt
F32 = DT.float32
F32R = DT.float32r
BF16 = DT.bfloat16
ALU = mybir.AluOpType
ACTF = mybir.ActivationFunctionType

S, D, NCORES = 8192, 1024, 8
SL = S // NCORES          # 1024 queries per core
P = 128                   # partitions
DC = D // P               # 8 contraction chunks
MC = SL // P              # 8 query chunks per core
CB = 512                  # key block size
NV = S // CB              # 16 key blocks
TC = CB // P              # 4 t-chunks per block
SCALE = 1.0 / 32.0        # 1/sqrt(D)
NEG_BIG = -1.0e30


def build_program(num_devices=NCORES):
    nc = bacc.Bacc("TRN2", target_bir_lowering=False, debug=False,
                   num_devices=num_devices)

    rt_p = nc.declare_dram_parameter("rt", [D, D], F32R, isOutput=False)
    et_p = nc.declare_dram_parameter("et", [D, D], F32R, isOutput=False)
    # X^T pre-blocked by key block: [NV, D, CB] flattened to [NV*D, CB]
    xtb_p = nc.declare_dram_parameter("xtb", [NV * D, CB], F32R,
                                      isOutput=False)
    xbf_p = nc.declare_dram_parameter("xbf", [S, D], BF16, isOutput=False)
    out_p = nc.declare_dram_parameter("out", [SL, D], F32, isOutput=True)

    with tile.TileContext(nc) as tc:
        with (
            tc.tile_pool(name="persist", bufs=1) as pers,
        ):
            tq = pers.tile([P, DC * SL], F32R, tag="tq")      # T^T, [c|m]
            oacc = pers.tile([P, MC * D], F32, tag="oacc")    # O accum per m
            ident_bf = pers.tile([P, P], BF16, tag="identbf")
            mst = [[pers.tile([P, 1], F32, tag=f"mst{m}_{j}",
                              name=f"mst{m}_{j}")
                    for j in range(2)] for m in range(MC)]
            sig = [pers.tile([P, 1], F32, tag=f"sig{m}", name=f"sig{m}")
                   for m in range(MC)]

            with tc.tile_pool(name="ident_tmp", bufs=1) as identp:
                ident32 = identp.tile([P, P], F32, tag="ident32")
                make_identity(nc, ident32[:])
                nc.vector.tensor_copy(ident_bf[:], ident32[:])
            nc.vector.memset(oacc[:], 0.0)
            for m in range(MC):
                nc.vector.memset(mst[m][0][:], NEG_BIG)
                nc.vector.memset(sig[m][:], 0.0)

            pid_sy = nc.sync.partition_id()
            pid_sc = nc.scalar.partition_id()
            pid_gp = nc.gpsimd.partition_id()

            with (
                tc.tile_pool(name="kt", bufs=2) as ktp,
                tc.tile_pool(name="xb", bufs=2) as xbp,
            ):
                def load_kt(j, eng, pid):
                    kt = ktp.tile([P, DC * CB], F32R, tag="kt", name="kt")
                    eng.dma_start(
                        kt.rearrange("p (k c) -> p k c", k=DC),
                        xtb_p[bass_mod.ds(
                            ((pid * 2 + j) % NV) * D, D), :]
                        .rearrange("(k p) c -> p k c", p=P))
                    return kt

                def load_xb(j):
                    xb = xbp.tile([P, TC * D], BF16, tag="xb", name="xb")
                    nc.gpsimd.dma_start(
                        xb.rearrange("p (k c) -> p k c", k=TC),
                        xbf_p[bass_mod.ds(
                            ((pid_gp * 2 + j) % NV) * CB, CB), :]
                        .rearrange("(k p) c -> p k c", p=P))
                    return xb

                # own key blocks double as X^T_own for the T^T projection
                xb0 = load_xb(0)
                xb1 = load_xb(1)

                # ------------ Phase A: W = R @ E^T, then T^T ------------
                with (
                    tc.tile_pool(name="pa", bufs=1) as pa,
                    tc.tile_pool(name="rt", bufs=2) as rtp,
                    tc.tile_pool(name="pa_ps", bufs=2, space="PSUM") as pa_ps,
                ):
                    et_sb = pa.tile([P, DC * D], F32R, tag="et")  # [d | j]
                    w_sb = pa.tile([P, DC * D], F32R, tag="w")    # [i | j]
                    et_r = et_sb.rearrange("p (k c) -> p k c", k=DC)
                    et_src = et_p.rearrange("(k p) c -> p k c", p=P)
                    for jh in range(2):
                        nc.scalar.dma_start(
                            et_r[:, :, jh * CB:(jh + 1) * CB],
                            et_src[:, :, jh * CB:(jh + 1) * CB])
                    kt1 = load_kt(1, nc.scalar, pid_sc)
                    rt_src = rt_p.rearrange("(k p) c -> p k c", p=P)

                    def load_rt(g):
                        rt_t = rtp.tile([P, DC * 2 * P], F32R, tag="rtg",
                                        name="rtg")
                        nc.sync.dma_start(
                            rt_t.rearrange("p (k c) -> p k c", k=DC),
                            rt_src[:, :, g * 2 * P:(g + 1) * 2 * P])
                        return rt_t

                    # W[i, j] = sum_d R^T[d, i] * E^T[d, j]
                    # rt streamed in i-pair slabs of [128, 8k x 256]
                    NG = DC // 2
                    rt_tiles = {0: load_rt(0), 1: load_rt(1)}
                    kt0 = None
                    for g in range(NG):
                        rt_t = rt_tiles.pop(g)
                        for i2 in range(2):
                            i = g * 2 + i2
                            for jh in range(2):
                                ps = pa_ps.tile([P, CB], F32, tag="proj")
                                for k in range(DC):
                                    nc.tensor.matmul(
                                        ps[:],
                                        rt_t[:, k * 2 * P + i2 * P:
                                             k * 2 * P + (i2 + 1) * P],
                                        et_sb[:, k * D + jh * CB:
                                              k * D + (jh + 1) * CB],
                                        start=(k == 0), stop=(k == DC - 1),
                                    )
                                nc.vector.tensor_copy(
                                    w_sb[:, i * D + jh * CB:
                                         i * D + (jh + 1) * CB],
                                    ps[:])
                        if g + 2 < NG:
                            rt_tiles[g + 2] = load_rt(g + 2)
                        elif g + 2 == NG:
                            # own kt block queues on sync after all rt slabs
                            kt0 = load_kt(0, nc.sync, pid_sy)

                    # T^T[c, m] = sum_d W[d, c] * X^T[d, m]
                    # moving operand comes straight from the own kt blocks
                    for mh, kt_own in ((0, kt0), (1, kt1)):
                        for c in range(DC):
                            ps = pa_ps.tile([P, CB], F32, tag="proj")
                            for k in range(DC):
                                nc.tensor.matmul(
                                    ps[:],
                                    w_sb[:, k * D + c * P:
                                         k * D + (c + 1) * P],
                                    kt_own[:, k * CB:(k + 1) * CB],
                                    start=(k == 0), stop=(k == DC - 1),
                                )
                            nc.vector.tensor_copy(
                                tq[:, c * SL + mh * CB:
                                   c * SL + (mh + 1) * CB],
                                ps[:])

                # ------------- Phase B: blocked attention ---------------
                # 16 key blocks of 512 in ring order starting at the own
                # blocks.  Software-pipelined: PE runs PV of a previous
                # block's m while DVE/ACT compute stats+exp of the current.
                self_attention_pools = (
                    tc.tile_pool(name="ph", bufs=4),
                    tc.tile_pool(name="pt", bufs=3),
                    tc.tile_pool(name="of", bufs=2),
                    tc.tile_pool(name="stats", bufs=6),
                    tc.tile_pool(name="s_ps", bufs=4, space="PSUM"),
                    tc.tile_pool(name="t_ps", bufs=2, space="PSUM"),
                    tc.tile_pool(name="o_ps", bufs=1, space="PSUM"),
                )
                with (
                    self_attention_pools[0] as php,
                    self_attention_pools[1] as ptp,
                    self_attention_pools[2] as ofp,
                    self_attention_pools[3] as stp,
                    self_attention_pools[4] as sps,
                    self_attention_pools[5] as tps,
                    self_attention_pools[6] as ops,
                ):
                    def flush_pe(pend):
                        ph, alpha, m, j, xb = pend
                        tp = tps.tile([P, CB], BF16, tag="tp", name="tp")
                        for tc_ in range(TC):
                            nc.tensor.transpose(
                                tp[:, tc_ * P:(tc_ + 1) * P],
                                ph[:, tc_ * P:(tc_ + 1) * P],
                                ident_bf[:],
                            )
                        pt = ptp.tile([P, CB], BF16, tag="pt", name="pt")
                        nc.scalar.copy(pt[:], tp[:])
                        o_part = ops.tile([P, D], F32, tag="opart",
                                          name="o_part")
                        for tc_ in range(TC):
                            for h in range(D // CB):
                                nc.tensor.matmul(
                                    o_part[:, h * CB:(h + 1) * CB],
                                    pt[:, tc_ * P:(tc_ + 1) * P],
                                    xb[:, tc_ * D + h * CB:
                                       tc_ * D + (h + 1) * CB],
                                    start=(tc_ == 0), stop=(tc_ == TC - 1),
                                )
                        return o_part

                    def flush_dve(pend, o_part):
                        ph, alpha, m, j, xb = pend
                        nc.vector.scalar_tensor_tensor(
                            oacc[:, m * D:(m + 1) * D],
                            oacc[:, m * D:(m + 1) * D],
                            alpha[:], o_part[:],
                            op0=ALU.mult, op1=ALU.add)
                        if j == NV - 1:
                            # finalize this m: divide by softmax sum, store
                            rcp = stp.tile([P, 1], F32, tag="rcp",
                                           name="rcp")
                            nc.vector.reciprocal(rcp[:], sig[m][:])
                            of = ofp.tile([P, D], F32, tag="ofin",
                                          name="ofin")
                            nc.vector.tensor_scalar_mul(
                                of[:], oacc[:, m * D:(m + 1) * D], rcp[:])
                            nc.sync.dma_start(out_p[m * P:(m + 1) * P, :],
                                              of[:])

                    pending = []
                    for j in range(NV):
                        if j == 0:
                            kt, xb = kt0, xb0
                        elif j == 1:
                            kt, xb = kt1, xb1
                        else:
                            kt = load_kt(j, nc.sync if j % 2 == 0
                                         else nc.scalar,
                                         pid_sy if j % 2 == 0 else pid_sc)
                            xb = load_xb(j)

                        for m in range(MC):
                            sh = sps.tile([P, CB], F32, tag="s", name="s")
                            for k in range(DC):
                                nc.tensor.matmul(
                                    sh[:],
                                    tq[:, k * SL + m * P:
                                       k * SL + (m + 1) * P],
                                    kt[:, k * CB:(k + 1) * CB],
                                    start=(k == 0), stop=(k == DC - 1),
                                )
                            mq = stp.tile([P, 1], F32, tag="mq", name="mq")
                            nc.vector.reduce_max(mq[:], sh[:],
                                                 axis=mybir.AxisListType.X)

                            # online softmax stats; mst ping-pongs on j
                            m_old = mst[m][j % 2]
                            mnew = mst[m][(j + 1) % 2]
                            nc.vector.tensor_max(mnew[:], m_old[:], mq[:])
                            nbias = stp.tile([P, 1], F32, tag="nbias",
                                             name="nbias")
                            nc.scalar.mul(nbias[:], mnew[:], -SCALE)
                            # alpha = exp((m_old - mnew)/32)
                            alpha = stp.tile([P, 1], F32, tag="alpha",
                                             name="alpha")
                            nc.scalar.activation(alpha[:], m_old[:],
                                                 ACTF.Exp,
                                                 bias=nbias[:], scale=SCALE)

                            # phat = exp(s/32 - mnew/32) in bf16; sums in sq
                            sq = stp.tile([P, 1], F32, tag="sq", name="sq")
                            ph = php.tile([P, CB], BF16, tag="ph",
                                          name="ph")
                            nc.scalar.activation(ph[:], sh[:], ACTF.Exp,
                                                 bias=nbias[:], scale=SCALE,
                                                 accum_out=sq[:])
                            nc.vector.scalar_tensor_tensor(
                                sig[m][:], sig[m][:], alpha[:], sq[:],
                                op0=ALU.mult, op1=ALU.add)

                            pending.append((ph, alpha, m, j, xb))
                            if len(pending) > 2:
                                pend_fl = pending.pop(0)
                                flush_dve(pend_fl, flush_pe(pend_fl))
                    for pend in pending:
                        flush_dve(pend, flush_pe(pend))

    nc.compile()
    return nc


_PROGRAM = None


def _get_program():
    global _PROGRAM
    if _PROGRAM is None:
        _PROGRAM = build_program()
    return _PROGRAM


def kernel(inputs, rotation_params, entangle_params, _trace=False):
    X = np.ascontiguousarray(np.asarray(inputs, dtype=np.float32))
    R = np.ascontiguousarray(np.asarray(rotation_params, dtype=np.float32))
    E = np.ascontiguousarray(np.asarray(entangle_params, dtype=np.float32))
    assert X.shape == (S, D) and R.shape == (D, D) and E.shape == (D, D)

    import ml_dtypes
    XT = np.ascontiguousarray(X.T)
    RT = np.ascontiguousarray(R.T)
    ET = np.ascontiguousarray(E.T)
    # X^T pre-blocked by key block: [NV, D, CB] -> [NV*D, CB]
    XTB = np.ascontiguousarray(
        XT.reshape(D, NV, CB).transpose(1, 0, 2)).reshape(NV * D, CB)
    Xbf = np.ascontiguousarray(X.astype(ml_dtypes.bfloat16))
    in_maps = []
    for i in range(NCORES):
        in_maps.append({
            "rt": RT,
            "et": ET,
            "xtb": XTB,
            "xbf": Xbf,
        })

    nc = _get_program()
    res = run_bass_kernel_spmd(nc, in_maps, list(range(NCORES)),
                               trace=_trace)
    out = np.concatenate([res.results[i]["out"] for i in range(NCORES)],
                         axis=0)
    if _trace:
        return out, res
    return out


# revision 17
# speedup vs baseline: 1.2435x; 1.0096x over previous
"""Trainium2 Bass kernel for ClassicalSelfAttention.

  out = softmax((X @ R) @ (X @ E).T / sqrt(D)) @ X,  X: (8192, 1024) fp32

Key identity: scores = (X R)(X E)^T = X (R E^T) X^T.  Each core computes
W = R @ E^T redundantly (27us of PE), projects its own query slice
(T^T with lhsT=W, rhs=X^T_own), and then the "keys" are just X^T itself
— which every core already holds in DRAM.  No collectives at all: the
attention loop streams X^T / X blocks straight from HBM.

Sequence-parallel over 8 NeuronCores: core i owns queries
[i*1024, (i+1)*1024).  Attention runs over 16 key blocks of 512, ring
order starting at the core's own two blocks (which double as the T^T
moving operand — no separate X^T_own load), with a standard online
softmax (per-query running max on DVE, exp on ACT, PV accumulated in
PSUM then merged into SBUF).  P^T for the PV matmul is produced by PE
transposes against a bf16 identity.

QK and the projections run in float32r (~13-bit mantissa at full PE
rate) — bf16 logits would flip argmaxes of this extremely peaked
softmax.  P is cast to bf16 by the exp activation; PV runs bf16.

Startup DMAs are split by first-consumer order (rt in i-pair slabs, et
in halves) across the two HWDGE queues so the first W matmul issues
early instead of waiting for whole-tensor loads.
"""
import numpy as np

import concourse.bass as bass_mod
import concourse.bacc as bacc
import concourse.mybir as mybir
from concourse import tile
from concourse.bass_utils import run_bass_kernel_spmd
from concourse.masks import make_identity

DT = mybir.dt
F32 = DT.float32
F32R = DT.float32r
BF16 = DT.bfloat16
ALU = mybir.AluOpType
ACTF = mybir.ActivationFunctionType

S, D, NCORES = 8192, 1024, 8
SL = S // NCORES          # 1024 queries per core
P = 128                   # partitions
DC = D // P               # 8 contraction chunks
MC = SL // P              # 8 query chunks per core
CB = 512                  # key block size
NV = S // CB              # 16 key blocks
TC = CB // P              # 4 t-chunks per block
SCALE = 1.0 / 32.0        # 1/sqrt(D)
NEG_BIG = -1.0e30


def build_program(num_devices=NCORES):
    nc = bacc.Bacc("TRN2", target_bir_lowering=False, debug=False,
                   num_devices=num_devices)

    rt_p = nc.declare_dram_parameter("rt", [D, D], F32R, isOutput=False)
    et_p = nc.declare_dram_parameter("et", [D, D], F32R, isOutput=False)
    # X^T pre-blocked by key block: [NV, D, CB] flattened to [NV*D, CB]
    xtb_p = nc.declare_dram_parameter("xtb", [NV * D, CB], F32R,
                                      isOutput=False)
    xbf_p = nc.declare_dram_parameter("xbf", [S, D], BF16, isOutput=False)
    out_p = nc.declare_dram_parameter("out", [SL, D], F32, isOutput=True)

    with tile.TileContext(nc) as tc:
        with (
            tc.tile_pool(name="persist", bufs=1) as pers,
        ):
            tq = pers.tile([P, DC * SL], F32R, tag="tq")      # T^T, [c|m]
            oacc = pers.tile([P, MC * D], F32, tag="oacc")    # O accum per m
            ident_bf = pers.tile([P, P], BF16, tag="identbf")
            mst = [[pers.tile([P, 1], F32, tag=f"mst{m}_{j}",
                              name=f"mst{m}_{j}")
                    for j in range(2)] for m in range(MC)]
            sig = [pers.tile([P, 1], F32, tag=f"sig{m}", name=f"sig{m}")
                   for m in range(MC)]

            with tc.tile_pool(name="ident_tmp", bufs=1) as identp:
                ident32 = identp.tile([P, P], F32, tag="ident32")
                make_identity(nc, ident32[:])
                nc.vector.tensor_copy(ident_bf[:], ident32[:])
            nc.vector.memset(oacc[:], 0.0)
            for m in range(MC):
                nc.vector.memset(mst[m][0][:], NEG_BIG)
                nc.vector.memset(sig[m][:], 0.0)

            pid_sy = nc.sync.partition_id()
            pid_sc = nc.scalar.partition_id()
            pid_gp = nc.gpsimd.partition_id()

            with (
                tc.tile_pool(name="kt", bufs=2) as ktp,
                tc.tile_pool(name="xb", bufs=2) as xbp,
            ):
                def load_kt(j, eng, pid):
                    kt = ktp.tile([P, DC * CB], F32R, tag="kt", name="kt")
                    eng.dma_start(
                        kt.rearrange("p (k c) -> p k c", k=DC),
                        xtb_p[bass_mod.ds(
                            ((pid * 2 + j) % NV) * D, D), :]
                        .rearrange("(k p) c -> p k c", p=P))
                    return kt

                def load_xb(j):
                    xb = xbp.tile([P, TC * D], BF16, tag="xb", name="xb")
                    nc.gpsimd.dma_start(
                        xb.rearrange("p (k c) -> p k c", k=TC),
                        xbf_p[bass_mod.ds(
                            ((pid_gp * 2 + j) % NV) * CB, CB), :]
                        .rearrange("(k p) c -> p k c", p=P))
                    return xb

                # own key blocks double as X^T_own for the T^T projection
                xb0 = load_xb(0)
                xb1 = load_xb(1)

                # ------------ Phase A: W = R @ E^T, then T^T ------------
                with (
                    tc.tile_pool(name="pa", bufs=1) as pa,
                    tc.tile_pool(name="rt", bufs=2) as rtp,
                    tc.tile_pool(name="pa_ps", bufs=2, space="PSUM") as pa_ps,
                ):
                    et_sb = pa.tile([P, DC * D], F32R, tag="et")  # [d | j]
                    w_sb = pa.tile([P, DC * D], F32R, tag="w")    # [i | j]
                    et_r = et_sb.rearrange("p (k c) -> p k c", k=DC)
                    et_src = et_p.rearrange("(k p) c -> p k c", p=P)
                    for jh in range(2):
                        nc.scalar.dma_start(
                            et_r[:, :, jh * CB:(jh + 1) * CB],
                            et_src[:, :, jh * CB:(jh + 1) * CB])
                    kt1 = load_kt(1, nc.scalar, pid_sc)
                    rt_src = rt_p.rearrange("(k p) c -> p k c", p=P)

                    def load_rt(g):
                        rt_t = rtp.tile([P, DC * 2 * P], F32R, tag="rtg",
                                        name="rtg")
                        nc.sync.dma_start(
                            rt_t.rearrange("p (k c) -> p k c", k=DC),
                            rt_src[:, :, g * 2 * P:(g + 1) * 2 * P])
                        return rt_t

                    # W[i, j] = sum_d R^T[d, i] * E^T[d, j]
                    # rt streamed in i-pair slabs of [128, 8k x 256]
                    NG = DC // 2
                    rt_tiles = {0: load_rt(0), 1: load_rt(1)}
                    kt0 = None
                    for g in range(NG):
                        rt_t = rt_tiles.pop(g)
                        for i2 in range(2):
                            i = g * 2 + i2
                            for jh in range(2):
                                ps = pa_ps.tile([P, CB], F32, tag="proj")
                                for k in range(DC):
                                    nc.tensor.matmul(
                                        ps[:],
                                        rt_t[:, k * 2 * P + i2 * P:
                                             k * 2 * P + (i2 + 1) * P],
                                        et_sb[:, k * D + jh * CB:
                                              k * D + (jh + 1) * CB],
                                        start=(k == 0), stop=(k == DC - 1),
                                    )
                                nc.vector.tensor_copy(
                                    w_sb[:, i * D + jh * CB:
                                         i * D + (jh + 1) * CB],
                                    ps[:])
                        if g + 2 < NG:
                            rt_tiles[g + 2] = load_rt(g + 2)
                        elif g + 2 == NG:
                            # own kt block queues on sync after all rt slabs
                            kt0 = load_kt(0, nc.sync, pid_sy)

                    # T^T[c, m] = sum_d W[d, c] * X^T[d, m]
                    # moving operand comes straight from the own kt blocks
                    for mh, kt_own in ((0, kt0), (1, kt1)):
                        for c in range(DC):
                            ps = pa_ps.tile([P, CB], F32, tag="proj")
                            for k in range(DC):
                                nc.tensor.matmul(
                                    ps[:],
                                    w_sb[:, k * D + c * P:
                                         k * D + (c + 1) * P],
                                    kt_own[:, k * CB:(k + 1) * CB],
                                    start=(k == 0), stop=(k == DC - 1),
                                )
                            nc.vector.tensor_copy(
                                tq[:, c * SL + mh * CB:
                                   c * SL + (mh + 1) * CB],
                                ps[:])

                # ------------- Phase B: blocked attention ---------------
                # 16 key blocks of 512 in ring order starting at the own
                # blocks.  Software-pipelined: PE runs PV of a previous
                # block's m while DVE/ACT compute stats+exp of the current.
                self_attention_pools = (
                    tc.tile_pool(name="ph", bufs=4),
                    tc.tile_pool(name="pt", bufs=3),
                    tc.tile_pool(name="of", bufs=2),
                    tc.tile_pool(name="stats", bufs=6),
                    tc.tile_pool(name="s_ps", bufs=4, space="PSUM"),
                    tc.tile_pool(name="t_ps", bufs=2, space="PSUM"),
                    tc.tile_pool(name="o_ps", bufs=1, space="PSUM"),
                )
                with (
                    self_attention_pools[0] as php,
                    self_attention_pools[1] as ptp,
                    self_attention_pools[2] as ofp,
                    self_attention_pools[3] as stp,
                    self_attention_pools[4] as sps,
                    self_attention_pools[5] as tps,
                    self_attention_pools[6] as ops,
                ):
                    def flush_dve(pend, o_part):
                        ph, alpha, m, j, xb, pt = pend
                        nc.vector.scalar_tensor_tensor(
                            oacc[:, m * D:(m + 1) * D],
                            oacc[:, m * D:(m + 1) * D],
                            alpha[:], o_part[:],
                            op0=ALU.mult, op1=ALU.add)
                        if j == NV - 1:
                            # finalize this m: divide by softmax sum, store
                            rcp = stp.tile([P, 1], F32, tag="rcp",
                                           name="rcp")
                            nc.vector.reciprocal(rcp[:], sig[m][:])
                            of = ofp.tile([P, D], F32, tag="ofin",
                                          name="ofin")
                            nc.vector.tensor_scalar_mul(
                                of[:], oacc[:, m * D:(m + 1) * D], rcp[:])
                            nc.sync.dma_start(out_p[m * P:(m + 1) * P, :],
                                              of[:])

                    def emit_step(tr, pv):
                        # Interleave the LDW-bound PE transposes of pending
                        # `tr` between the PV matmuls of pending `pv` so the
                        # transpose weight loads hide under the 213ns PV MMs.
                        tp = None
                        if tr is not None:
                            tp = tps.tile([P, CB], BF16, tag="tp", name="tp")
                        o_part = None
                        if pv is not None:
                            o_part = ops.tile([P, D], F32, tag="opart",
                                              name="o_part")
                        for tc_ in range(TC):
                            if tr is not None:
                                nc.tensor.transpose(
                                    tp[:, tc_ * P:(tc_ + 1) * P],
                                    tr[0][:, tc_ * P:(tc_ + 1) * P],
                                    ident_bf[:],
                                )
                            if pv is not None:
                                pt, xb = pv[5], pv[4]
                                for h in range(D // CB):
                                    nc.tensor.matmul(
                                        o_part[:, h * CB:(h + 1) * CB],
                                        pt[:, tc_ * P:(tc_ + 1) * P],
                                        xb[:, tc_ * D + h * CB:
                                           tc_ * D + (h + 1) * CB],
                                        start=(tc_ == 0),
                                        stop=(tc_ == TC - 1),
                                    )
                        if tr is not None:
                            pt_new = ptp.tile([P, CB], BF16, tag="pt",
                                              name="pt")
                            nc.scalar.copy(pt_new[:], tp[:])
                            tr[5] = pt_new
                        if pv is not None:
                            flush_dve(pv, o_part)

                    pend_s = []   # stats done, needs transpose
                    pend_t = []   # transposed, needs PV
                    for j in range(NV):
                        if j == 0:
                            kt, xb = kt0, xb0
                        elif j == 1:
                            kt, xb = kt1, xb1
                        else:
                            kt = load_kt(j, nc.sync if j % 2 == 0
                                         else nc.scalar,
                                         pid_sy if j % 2 == 0 else pid_sc)
                            xb = load_xb(j)

                        for m in range(MC):
                            sh = sps.tile([P, CB], F32, tag="s", name="s")
                            for k in range(DC):
                                nc.tensor.matmul(
                                    sh[:],
                                    tq[:, k * SL + m * P:
                                       k * SL + (m + 1) * P],
                                    kt[:, k * CB:(k + 1) * CB],
                                    start=(k == 0), stop=(k == DC - 1),
                                )
                            mq = stp.tile([P, 1], F32, tag="mq", name="mq")
                            nc.vector.reduce_max(mq[:], sh[:],
                                                 axis=mybir.AxisListType.X)

                            # online softmax stats; mst ping-pongs on j
                            m_old = mst[m][j % 2]
                            mnew = mst[m][(j + 1) % 2]
                            nc.vector.tensor_max(mnew[:], m_old[:], mq[:])
                            nbias = stp.tile([P, 1], F32, tag="nbias",
                                             name="nbias")
                            nc.scalar.mul(nbias[:], mnew[:], -SCALE)
                            # alpha = exp((m_old - mnew)/32)
                            alpha = stp.tile([P, 1], F32, tag="alpha",
                                             name="alpha")
                            nc.scalar.activation(alpha[:], m_old[:],
                                                 ACTF.Exp,
                                                 bias=nbias[:], scale=SCALE)

                            # phat = exp(s/32 - mnew/32) in bf16; sums in sq
                            sq = stp.tile([P, 1], F32, tag="sq", name="sq")
                            ph = php.tile([P, CB], BF16, tag="ph",
                                          name="ph")
                            nc.scalar.activation(ph[:], sh[:], ACTF.Exp,
                                                 bias=nbias[:], scale=SCALE,
                                                 accum_out=sq[:])
                            nc.vector.scalar_tensor_tensor(
                                sig[m][:], sig[m][:], alpha[:], sq[:],
                                op0=ALU.mult, op1=ALU.add)

                            pend_s.append([ph, alpha, m, j, xb, None])
                            if len(pend_s) >= 2:
                                tr = pend_s.pop(0)
                                pv = pend_t.pop(0) if pend_t else None
                                emit_step(tr, pv)
                                pend_t.append(tr)
                    while pend_s or pend_t:
                        tr = pend_s.pop(0) if pend_s else None
                        pv = pend_t.pop(0) if pend_t else None
                        emit_step(tr, pv)
                        if tr is not None:
                            pend_t.append(tr)

    nc.compile()
    return nc


_PROGRAM = None


def _get_program():
    global _PROGRAM
    if _PROGRAM is None:
        _PROGRAM = build_program()
    return _PROGRAM


def kernel(inputs, rotation_params, entangle_params, _trace=False):
    X = np.ascontiguousarray(np.asarray(inputs, dtype=np.float32))
    R = np.ascontiguousarray(np.asarray(rotation_params, dtype=np.float32))
    E = np.ascontiguousarray(np.asarray(entangle_params, dtype=np.float32))
    assert X.shape == (S, D) and R.shape == (D, D) and E.shape == (D, D)

    import ml_dtypes
    XT = np.ascontiguousarray(X.T)
    RT = np.ascontiguousarray(R.T)
    ET = np.ascontiguousarray(E.T)
    # X^T pre-blocked by key block: [NV, D, CB] -> [NV*D, CB]
    XTB = np.ascontiguousarray(
        XT.reshape(D, NV, CB).transpose(1, 0, 2)).reshape(NV * D, CB)
    Xbf = np.ascontiguousarray(X.astype(ml_dtypes.bfloat16))
    in_maps = []
    for i in range(NCORES):
        in_maps.append({
            "rt": RT,
            "et": ET,
            "xtb": XTB,
            "xbf": Xbf,
        })

    nc = _get_program()
    res = run_bass_kernel_spmd(nc, in_maps, list(range(NCORES)),
                               trace=_trace)
    out = np.concatenate([res.results[i]["out"] for i in range(NCORES)],
                         axis=0)
    if _trace:
        return out, res
    return out


# revision 22
# speedup vs baseline: 1.2731x; 1.0238x over previous
"""Trainium2 Bass kernel for ClassicalSelfAttention.

  out = softmax((X @ R) @ (X @ E).T / sqrt(D)) @ X,  X: (8192, 1024) fp32

Key identity: scores = (X R)(X E)^T = X (R E^T) X^T.  Each core computes
W = R @ E^T redundantly (27us of PE), projects its own query slice
(T^T with lhsT=W, rhs=X^T_own), and then the "keys" are just X^T itself
— which every core already holds in DRAM.  No collectives at all: the
attention loop streams X^T / X blocks straight from HBM.

Sequence-parallel over 8 NeuronCores: core i owns queries
[i*1024, (i+1)*1024).  Attention runs over 16 key blocks of 512, ring
order starting at the core's own two blocks (which double as the T^T
moving operand — no separate X^T_own load), with a standard online
softmax (per-query running max on DVE, exp on ACT, PV accumulated in
PSUM then merged into SBUF).  P^T for the PV matmul is produced by PE
transposes against a bf16 identity.

QK and the projections run in float32r (~13-bit mantissa at full PE
rate) — bf16 logits would flip argmaxes of this extremely peaked
softmax.  P is cast to bf16 by the exp activation; PV runs bf16.

Startup DMAs are split by first-consumer order (rt in i-pair slabs, et
in halves) across the two HWDGE queues so the first W matmul issues
early instead of waiting for whole-tensor loads.
"""
import numpy as np

import concourse.bass as bass_mod
import concourse.bacc as bacc
import concourse.mybir as mybir
from concourse import tile
from concourse.bass_utils import run_bass_kernel_spmd
from concourse.masks import make_identity

DT = mybir.dt
F32 = DT.float32
F32R = DT.float32r
BF16 = DT.bfloat16
ALU = mybir.AluOpType
ACTF = mybir.ActivationFunctionType

S, D, NCORES = 8192, 1024, 8
SL = S // NCORES          # 1024 queries per core
P = 128                   # partitions
DC = D // P               # 8 contraction chunks
MC = SL // P              # 8 query chunks per core
CB = 512                  # key block size
NV = S // CB              # 16 key blocks
TC = CB // P              # 4 t-chunks per block
SCALE = 1.0 / 32.0        # 1/sqrt(D)
NEG_BIG = -1.0e30


def build_program(num_devices=NCORES):
    nc = bacc.Bacc("TRN2", target_bir_lowering=False, debug=False,
                   num_devices=num_devices)

    rt_p = nc.declare_dram_parameter("rt", [D, D], F32R, isOutput=False)
    et_p = nc.declare_dram_parameter("et", [D, D], F32R, isOutput=False)
    # X^T pre-blocked by key block: [NV, D, CB] flattened to [NV*D, CB]
    xtb_p = nc.declare_dram_parameter("xtb", [NV * D, CB], F32R,
                                      isOutput=False)
    xbf_p = nc.declare_dram_parameter("xbf", [S, D], BF16, isOutput=False)
    out_p = nc.declare_dram_parameter("out", [SL, D], F32, isOutput=True)

    with tile.TileContext(nc) as tc:
        with (
            tc.tile_pool(name="persist", bufs=1) as pers,
        ):
            tq = pers.tile([P, DC * SL], F32R, tag="tq")      # T^T, [c|m]
            oacc = pers.tile([P, MC * D], F32, tag="oacc")    # O accum per m
            ident_bf = pers.tile([P, P], BF16, tag="identbf")
            mst = [[pers.tile([P, 1], F32, tag=f"mst{m}_{j}",
                              name=f"mst{m}_{j}")
                    for j in range(2)] for m in range(MC)]
            sig = [pers.tile([P, 1], F32, tag=f"sig{m}", name=f"sig{m}")
                   for m in range(MC)]

            with tc.tile_pool(name="ident_tmp", bufs=1) as identp:
                ident32 = identp.tile([P, P], F32, tag="ident32")
                make_identity(nc, ident32[:])
                nc.vector.tensor_copy(ident_bf[:], ident32[:])
            nc.vector.memset(oacc[:], 0.0)
            for m in range(MC):
                nc.vector.memset(mst[m][0][:], NEG_BIG)
                nc.vector.memset(sig[m][:], 0.0)

            pid_sy = nc.sync.partition_id()
            pid_sc = nc.scalar.partition_id()
            pid_gp = nc.gpsimd.partition_id()

            with (
                tc.tile_pool(name="kt", bufs=2) as ktp,
                tc.tile_pool(name="xb", bufs=2) as xbp,
            ):
                def load_kt(j, eng, pid):
                    kt = ktp.tile([P, DC * CB], F32R, tag="kt", name="kt")
                    eng.dma_start(
                        kt.rearrange("p (k c) -> p k c", k=DC),
                        xtb_p[bass_mod.ds(
                            ((pid * 2 + j) % NV) * D, D), :]
                        .rearrange("(k p) c -> p k c", p=P))
                    return kt

                def load_xb(j):
                    xb = xbp.tile([P, TC * D], BF16, tag="xb", name="xb")
                    nc.gpsimd.dma_start(
                        xb.rearrange("p (k c) -> p k c", k=TC),
                        xbf_p[bass_mod.ds(
                            ((pid_gp * 2 + j) % NV) * CB, CB), :]
                        .rearrange("(k p) c -> p k c", p=P))
                    return xb

                # own key blocks double as X^T_own for the T^T projection
                xb0 = load_xb(0)
                xb1 = load_xb(1)

                # ------------ Phase A: W = R @ E^T, then T^T ------------
                with (
                    tc.tile_pool(name="pa", bufs=1) as pa,
                    tc.tile_pool(name="rt", bufs=3) as rtp,
                    tc.tile_pool(name="pa_ps", bufs=2, space="PSUM") as pa_ps,
                ):
                    et_sb = pa.tile([P, DC * D], F32R, tag="et")  # [d | j]
                    w_sb = pa.tile([P, DC * D], F32R, tag="w")    # [i | j]
                    et_r = et_sb.rearrange("p (k c) -> p k c", k=DC)
                    et_src = et_p.rearrange("(k p) c -> p k c", p=P)
                    for jh in range(2):
                        nc.scalar.dma_start(
                            et_r[:, :, jh * CB:(jh + 1) * CB],
                            et_src[:, :, jh * CB:(jh + 1) * CB])
                    kt1 = load_kt(1, nc.scalar, pid_sc)
                    rt_src = rt_p.rearrange("(k p) c -> p k c", p=P)

                    def load_rt(g):
                        rt_t = rtp.tile([P, DC * 2 * P], F32R, tag="rtg",
                                        name="rtg")
                        nc.sync.dma_start(
                            rt_t.rearrange("p (k c) -> p k c", k=DC),
                            rt_src[:, :, g * 2 * P:(g + 1) * 2 * P])
                        return rt_t

                    # W[i, j] = sum_d R^T[d, i] * E^T[d, j]
                    # rt streamed in i-pair slabs of [128, 8k x 256]
                    NG = DC // 2
                    rt_tiles = {0: load_rt(0), 1: load_rt(1)}
                    kt0 = None
                    for g in range(NG):
                        rt_t = rt_tiles.pop(g)
                        for i2 in range(2):
                            i = g * 2 + i2
                            for jh in range(2):
                                ps = pa_ps.tile([P, CB], F32, tag="proj")
                                for k in range(DC):
                                    nc.tensor.matmul(
                                        ps[:],
                                        rt_t[:, k * 2 * P + i2 * P:
                                             k * 2 * P + (i2 + 1) * P],
                                        et_sb[:, k * D + jh * CB:
                                              k * D + (jh + 1) * CB],
                                        start=(k == 0), stop=(k == DC - 1),
                                    )
                                nc.vector.tensor_copy(
                                    w_sb[:, i * D + jh * CB:
                                         i * D + (jh + 1) * CB],
                                    ps[:])
                        if g + 2 < NG:
                            rt_tiles[g + 2] = load_rt(g + 2)
                        elif g + 2 == NG:
                            # own kt block queues on sync after all rt slabs
                            kt0 = load_kt(0, nc.sync, pid_sy)

                    # T^T[c, m] = sum_d W[d, c] * X^T[d, m]
                    # moving operand comes straight from the own kt blocks
                    for mh, kt_own in ((0, kt0), (1, kt1)):
                        for c in range(DC):
                            ps = pa_ps.tile([P, CB], F32, tag="proj")
                            for k in range(DC):
                                nc.tensor.matmul(
                                    ps[:],
                                    w_sb[:, k * D + c * P:
                                         k * D + (c + 1) * P],
                                    kt_own[:, k * CB:(k + 1) * CB],
                                    start=(k == 0), stop=(k == DC - 1),
                                )
                            nc.vector.tensor_copy(
                                tq[:, c * SL + mh * CB:
                                   c * SL + (mh + 1) * CB],
                                ps[:])

                # ------------- Phase B: blocked attention ---------------
                # 16 key blocks of 512 in ring order starting at the own
                # blocks.  Software-pipelined: PE runs PV of a previous
                # block's m while DVE/ACT compute stats+exp of the current.
                self_attention_pools = (
                    tc.tile_pool(name="ph", bufs=4),
                    tc.tile_pool(name="pt", bufs=3),
                    tc.tile_pool(name="of", bufs=2),
                    tc.tile_pool(name="stats", bufs=6),
                    tc.tile_pool(name="s_ps", bufs=3, space="PSUM"),
                    tc.tile_pool(name="t_ps", bufs=2, space="PSUM"),
                    tc.tile_pool(name="o_ps", bufs=3, space="PSUM"),
                )
                with (
                    self_attention_pools[0] as php,
                    self_attention_pools[1] as ptp,
                    self_attention_pools[2] as ofp,
                    self_attention_pools[3] as stp,
                    self_attention_pools[4] as sps,
                    self_attention_pools[5] as tps,
                    self_attention_pools[6] as ops,
                ):
                    NH = D // CB  # PV output halves (separate PSUM banks)
                    def flush_dve(pend, o_halves):
                        ph, alpha, m, j, xb, pt = pend
                        for h, o_h in enumerate(o_halves):
                            nc.vector.scalar_tensor_tensor(
                                oacc[:, m * D + h * CB:
                                     m * D + (h + 1) * CB],
                                oacc[:, m * D + h * CB:
                                     m * D + (h + 1) * CB],
                                alpha[:], o_h[:],
                                op0=ALU.mult, op1=ALU.add)
                        if j == NV - 1:
                            # finalize this m: divide by softmax sum, store
                            rcp = stp.tile([P, 1], F32, tag="rcp",
                                           name="rcp")
                            nc.vector.reciprocal(rcp[:], sig[m][:])
                            of = ofp.tile([P, D], F32, tag="ofin",
                                          name="ofin")
                            nc.vector.tensor_scalar_mul(
                                of[:], oacc[:, m * D:(m + 1) * D], rcp[:])
                            nc.sync.dma_start(out_p[m * P:(m + 1) * P, :],
                                              of[:])

                    def emit_step(tr, pv):
                        # Interleave the LDW-bound PE transposes of pending
                        # `tr` between the PV matmuls of pending `pv` so the
                        # transpose weight loads hide under the 213ns PV MMs.
                        tp = None
                        if tr is not None:
                            tp = tps.tile([P, CB], BF16, tag="tp", name="tp")
                        o_halves = None
                        if pv is not None:
                            o_halves = [ops.tile([P, CB], F32, tag="opart",
                                                 name="o_part")
                                        for _ in range(NH)]
                        for tc_ in range(TC):
                            if tr is not None:
                                nc.tensor.transpose(
                                    tp[:, tc_ * P:(tc_ + 1) * P],
                                    tr[0][:, tc_ * P:(tc_ + 1) * P],
                                    ident_bf[:],
                                )
                            if pv is not None:
                                pt, xb = pv[5], pv[4]
                                for h in range(NH):
                                    nc.tensor.matmul(
                                        o_halves[h][:],
                                        pt[:, tc_ * P:(tc_ + 1) * P],
                                        xb[:, tc_ * D + h * CB:
                                           tc_ * D + (h + 1) * CB],
                                        start=(tc_ == 0),
                                        stop=(tc_ == TC - 1),
                                    )
                        if tr is not None:
                            pt_new = ptp.tile([P, CB], BF16, tag="pt",
                                              name="pt")
                            nc.scalar.copy(pt_new[:], tp[:])
                            tr[5] = pt_new
                        if pv is not None:
                            flush_dve(pv, o_halves)

                    pend_s = []   # stats done, needs transpose
                    pend_t = []   # transposed, needs PV
                    for j in range(NV):
                        if j == 0:
                            kt, xb = kt0, xb0
                        elif j == 1:
                            kt, xb = kt1, xb1
                        else:
                            kt = load_kt(j, nc.sync if j % 2 == 0
                                         else nc.scalar,
                                         pid_sy if j % 2 == 0 else pid_sc)
                            xb = load_xb(j)

                        for m in range(MC):
                            sh = sps.tile([P, CB], F32, tag="s", name="s")
                            for k in range(DC):
                                nc.tensor.matmul(
                                    sh[:],
                                    tq[:, k * SL + m * P:
                                       k * SL + (m + 1) * P],
                                    kt[:, k * CB:(k + 1) * CB],
                                    start=(k == 0), stop=(k == DC - 1),
                                )
                            mq = stp.tile([P, 1], F32, tag="mq", name="mq")
                            nc.vector.reduce_max(mq[:], sh[:],
                                                 axis=mybir.AxisListType.X)

                            # online softmax stats; mst ping-pongs on j
                            m_old = mst[m][j % 2]
                            mnew = mst[m][(j + 1) % 2]
                            nc.vector.tensor_max(mnew[:], m_old[:], mq[:])
                            nbias = stp.tile([P, 1], F32, tag="nbias",
                                             name="nbias")
                            nc.scalar.mul(nbias[:], mnew[:], -SCALE)
                            # alpha = exp((m_old - mnew)/32)
                            alpha = stp.tile([P, 1], F32, tag="alpha",
                                             name="alpha")
                            nc.scalar.activation(alpha[:], m_old[:],
                                                 ACTF.Exp,
                                                 bias=nbias[:], scale=SCALE)

                            # phat = exp(s/32 - mnew/32) in bf16; sums in sq
                            sq = stp.tile([P, 1], F32, tag="sq", name="sq")
                            ph = php.tile([P, CB], BF16, tag="ph",
                                          name="ph")
                            nc.scalar.activation(ph[:], sh[:], ACTF.Exp,
                                                 bias=nbias[:], scale=SCALE,
                                                 accum_out=sq[:])
                            nc.vector.scalar_tensor_tensor(
                                sig[m][:], sig[m][:], alpha[:], sq[:],
                                op0=ALU.mult, op1=ALU.add)

                            pend_s.append([ph, alpha, m, j, xb, None])
                            if len(pend_s) >= 2:
                                tr = pend_s.pop(0)
                                pv = pend_t.pop(0) if pend_t else None
                                emit_step(tr, pv)
                                pend_t.append(tr)
                    while pend_s or pend_t:
                        tr = pend_s.pop(0) if pend_s else None
                        pv = pend_t.pop(0) if pend_t else None
                        emit_step(tr, pv)
                        if tr is not None:
                            pend_t.append(tr)

    nc.compile()
    return nc


_PROGRAM = None


def _get_program():
    global _PROGRAM
    if _PROGRAM is None:
        _PROGRAM = build_program()
    return _PROGRAM


def kernel(inputs, rotation_params, entangle_params, _trace=False):
    X = np.ascontiguousarray(np.asarray(inputs, dtype=np.float32))
    R = np.ascontiguousarray(np.asarray(rotation_params, dtype=np.float32))
    E = np.ascontiguousarray(np.asarray(entangle_params, dtype=np.float32))
    assert X.shape == (S, D) and R.shape == (D, D) and E.shape == (D, D)

    import ml_dtypes
    XT = np.ascontiguousarray(X.T)
    RT = np.ascontiguousarray(R.T)
    ET = np.ascontiguousarray(E.T)
    # X^T pre-blocked by key block: [NV, D, CB] -> [NV*D, CB]
    XTB = np.ascontiguousarray(
        XT.reshape(D, NV, CB).transpose(1, 0, 2)).reshape(NV * D, CB)
    Xbf = np.ascontiguousarray(X.astype(ml_dtypes.bfloat16))
    in_maps = []
    for i in range(NCORES):
        in_maps.append({
            "rt": RT,
            "et": ET,
            "xtb": XTB,
            "xbf": Xbf,
        })

    nc = _get_program()
    res = run_bass_kernel_spmd(nc, in_maps, list(range(NCORES)),
                               trace=_trace)
    out = np.concatenate([res.results[i]["out"] for i in range(NCORES)],
                         axis=0)
    if _trace:
        return out, res
    return out


# revision 26
# speedup vs baseline: 1.2944x; 1.0168x over previous
"""Trainium2 Bass kernel for ClassicalSelfAttention.

  out = softmax((X @ R) @ (X @ E).T / sqrt(D)) @ X,  X: (8192, 1024) fp32

Key identity: scores = (X R)(X E)^T = X (R E^T) X^T.  Each core computes
W = R @ E^T redundantly (27us of PE), projects its own query slice
(T^T with lhsT=W, rhs=X^T_own), and then the "keys" are just X^T itself
— which every core already holds in DRAM.  No collectives at all: the
attention loop streams X^T / X blocks straight from HBM.

Sequence-parallel over 8 NeuronCores: core i owns queries
[i*1024, (i+1)*1024).  Attention runs over 16 key blocks of 512, ring
order starting at the core's own two blocks (which double as the T^T
moving operand — no separate X^T_own load), with a standard online
softmax (per-query running max on DVE, exp on ACT, PV accumulated in
PSUM then merged into SBUF).  P^T for the PV matmul is produced by PE
transposes against a bf16 identity.

QK and the projections run in float32r (~13-bit mantissa at full PE
rate) — bf16 logits would flip argmaxes of this extremely peaked
softmax.  P is cast to bf16 by the exp activation; PV runs bf16.

Startup DMAs are split by first-consumer order (rt in i-pair slabs, et
in halves) across the two HWDGE queues so the first W matmul issues
early instead of waiting for whole-tensor loads.
"""
import numpy as np

import concourse.bass as bass_mod
import concourse.bacc as bacc
import concourse.mybir as mybir
from concourse import tile
from concourse.bass_utils import run_bass_kernel_spmd
from concourse.masks import make_identity

DT = mybir.dt
F32 = DT.float32
F32R = DT.float32r
BF16 = DT.bfloat16
F16 = DT.float16
ALU = mybir.AluOpType
ACTF = mybir.ActivationFunctionType

S, D, NCORES = 8192, 1024, 8
SL = S // NCORES          # 1024 queries per core
P = 128                   # partitions
DC = D // P               # 8 contraction chunks
MC = SL // P              # 8 query chunks per core
CB = 512                  # key block size
NV = S // CB              # 16 key blocks
TC = CB // P              # 4 t-chunks per block
SCALE = 1.0 / 32.0        # 1/sqrt(D)
NEG_BIG = -1.0e30


def build_program(num_devices=NCORES):
    nc = bacc.Bacc("TRN2", target_bir_lowering=False, debug=False,
                   num_devices=num_devices)

    rt_p = nc.declare_dram_parameter("rt", [D, D], F32R, isOutput=False)
    et_p = nc.declare_dram_parameter("et", [D, D], F32R, isOutput=False)
    # X^T pre-blocked by key block: [NV, D, CB] flattened to [NV*D, CB]
    xtb_p = nc.declare_dram_parameter("xtb", [NV * D, CB], F16,
                                      isOutput=False)
    # own X^T slice in f32r for the T^T projection (Q side stays clean)
    xt_p = nc.declare_dram_parameter("xt", [D, SL], F32R, isOutput=False)
    xbf_p = nc.declare_dram_parameter("xbf", [S, D], BF16, isOutput=False)
    out_p = nc.declare_dram_parameter("out", [SL, D], F32, isOutput=True)

    with tile.TileContext(nc) as tc:
        with (
            tc.tile_pool(name="persist", bufs=1) as pers,
        ):
            tq = pers.tile([P, DC * SL], F16, tag="tq")       # T^T, [c|m]
            oacc = pers.tile([P, MC * D], F32, tag="oacc")    # O accum per m
            ident_bf = pers.tile([P, P], BF16, tag="identbf")
            mst = [[pers.tile([P, 1], F32, tag=f"mst{m}_{j}",
                              name=f"mst{m}_{j}")
                    for j in range(2)] for m in range(MC)]
            sig = [pers.tile([P, 1], F32, tag=f"sig{m}", name=f"sig{m}")
                   for m in range(MC)]

            with tc.tile_pool(name="ident_tmp", bufs=1) as identp:
                ident32 = identp.tile([P, P], F32, tag="ident32")
                make_identity(nc, ident32[:])
                nc.vector.tensor_copy(ident_bf[:], ident32[:])
            nc.vector.memset(oacc[:], 0.0)
            for m in range(MC):
                nc.vector.memset(mst[m][0][:], NEG_BIG)
                nc.vector.memset(sig[m][:], 0.0)

            pid_sy = nc.sync.partition_id()
            pid_sc = nc.scalar.partition_id()
            pid_gp = nc.gpsimd.partition_id()

            with (
                tc.tile_pool(name="kt", bufs=2) as ktp,
                tc.tile_pool(name="xb", bufs=2) as xbp,
            ):
                def load_kt(j, eng, pid):
                    kt = ktp.tile([P, DC * CB], F16, tag="kt", name="kt")
                    eng.dma_start(
                        kt.rearrange("p (k c) -> p k c", k=DC),
                        xtb_p[bass_mod.ds(
                            ((pid * 2 + j) % NV) * D, D), :]
                        .rearrange("(k p) c -> p k c", p=P))
                    return kt

                def load_xb(j):
                    xb = xbp.tile([P, TC * D], BF16, tag="xb", name="xb")
                    nc.gpsimd.dma_start(
                        xb.rearrange("p (k c) -> p k c", k=TC),
                        xbf_p[bass_mod.ds(
                            ((pid_gp * 2 + j) % NV) * CB, CB), :]
                        .rearrange("(k p) c -> p k c", p=P))
                    return xb

                # own key blocks double as X^T_own for the T^T projection
                xb0 = load_xb(0)
                xb1 = load_xb(1)

                # ------------ Phase A: W = R @ E^T, then T^T ------------
                with (
                    tc.tile_pool(name="pa", bufs=1) as pa,
                    tc.tile_pool(name="rt", bufs=3) as rtp,
                    tc.tile_pool(name="pa_ps", bufs=2, space="PSUM") as pa_ps,
                ):
                    et_sb = pa.tile([P, DC * D], F32R, tag="et")  # [d | j]
                    w_sb = pa.tile([P, DC * D], F32R, tag="w")    # [i | j]
                    xt_sb = pa.tile([P, DC * SL], F32R, tag="xt")  # [d | m]
                    et_r = et_sb.rearrange("p (k c) -> p k c", k=DC)
                    et_src = et_p.rearrange("(k p) c -> p k c", p=P)
                    for jh in range(2):
                        nc.scalar.dma_start(
                            et_r[:, :, jh * CB:(jh + 1) * CB],
                            et_src[:, :, jh * CB:(jh + 1) * CB])
                    xt_r = xt_sb.rearrange("p (k c) -> p k c", k=DC)
                    xt_src = xt_p.rearrange("(k p) c -> p k c", p=P)
                    nc.scalar.dma_start(xt_r[:, :, CB:2 * CB],
                                        xt_src[:, :, CB:2 * CB])
                    kt1 = load_kt(1, nc.scalar, pid_sc)
                    rt_src = rt_p.rearrange("(k p) c -> p k c", p=P)

                    def load_rt(g):
                        rt_t = rtp.tile([P, DC * 2 * P], F32R, tag="rtg",
                                        name="rtg")
                        nc.sync.dma_start(
                            rt_t.rearrange("p (k c) -> p k c", k=DC),
                            rt_src[:, :, g * 2 * P:(g + 1) * 2 * P])
                        return rt_t

                    # W[i, j] = sum_d R^T[d, i] * E^T[d, j]
                    # rt streamed in i-pair slabs of [128, 8k x 256]
                    NG = DC // 2
                    rt_tiles = {0: load_rt(0), 1: load_rt(1)}
                    kt0 = None
                    for g in range(NG):
                        rt_t = rt_tiles.pop(g)
                        for i2 in range(2):
                            i = g * 2 + i2
                            for jh in range(2):
                                ps = pa_ps.tile([P, CB], F32, tag="proj")
                                for k in range(DC):
                                    nc.tensor.matmul(
                                        ps[:],
                                        rt_t[:, k * 2 * P + i2 * P:
                                             k * 2 * P + (i2 + 1) * P],
                                        et_sb[:, k * D + jh * CB:
                                              k * D + (jh + 1) * CB],
                                        start=(k == 0), stop=(k == DC - 1),
                                    )
                                nc.vector.tensor_copy(
                                    w_sb[:, i * D + jh * CB:
                                         i * D + (jh + 1) * CB],
                                    ps[:])
                        if g + 2 < NG:
                            rt_tiles[g + 2] = load_rt(g + 2)
                        elif g + 2 == NG:
                            # own X^T half + kt block queue after rt slabs
                            nc.sync.dma_start(xt_r[:, :, 0:CB],
                                              xt_src[:, :, 0:CB])
                            kt0 = load_kt(0, nc.sync, pid_sy)

                    # T^T[c, m] = sum_d W[d, c] * X^T[d, m]
                    for mh in range(2):
                        for c in range(DC):
                            ps = pa_ps.tile([P, CB], F32, tag="proj")
                            for k in range(DC):
                                nc.tensor.matmul(
                                    ps[:],
                                    w_sb[:, k * D + c * P:
                                         k * D + (c + 1) * P],
                                    xt_sb[:, k * SL + mh * CB:
                                          k * SL + (mh + 1) * CB],
                                    start=(k == 0), stop=(k == DC - 1),
                                )
                            nc.vector.tensor_copy(
                                tq[:, c * SL + mh * CB:
                                   c * SL + (mh + 1) * CB],
                                ps[:])

                # ------------- Phase B: blocked attention ---------------
                # 16 key blocks of 512 in ring order starting at the own
                # blocks.  Software-pipelined: PE runs PV of a previous
                # block's m while DVE/ACT compute stats+exp of the current.
                self_attention_pools = (
                    tc.tile_pool(name="ph", bufs=4),
                    tc.tile_pool(name="pt", bufs=3),
                    tc.tile_pool(name="of", bufs=2),
                    tc.tile_pool(name="stats", bufs=6),
                    tc.tile_pool(name="s_ps", bufs=3, space="PSUM"),
                    tc.tile_pool(name="t_ps", bufs=2, space="PSUM"),
                    tc.tile_pool(name="o_ps", bufs=3, space="PSUM"),
                )
                with (
                    self_attention_pools[0] as php,
                    self_attention_pools[1] as ptp,
                    self_attention_pools[2] as ofp,
                    self_attention_pools[3] as stp,
                    self_attention_pools[4] as sps,
                    self_attention_pools[5] as tps,
                    self_attention_pools[6] as ops,
                ):
                    NH = D // CB  # PV output halves (separate PSUM banks)
                    def flush_dve(pend, o_halves):
                        ph, alpha, m, j, xb, pt = pend
                        for h, o_h in enumerate(o_halves):
                            nc.vector.scalar_tensor_tensor(
                                oacc[:, m * D + h * CB:
                                     m * D + (h + 1) * CB],
                                oacc[:, m * D + h * CB:
                                     m * D + (h + 1) * CB],
                                alpha[:], o_h[:],
                                op0=ALU.mult, op1=ALU.add)
                        if j == NV - 1:
                            # finalize this m: divide by softmax sum, store
                            rcp = stp.tile([P, 1], F32, tag="rcp",
                                           name="rcp")
                            nc.vector.reciprocal(rcp[:], sig[m][:])
                            of = ofp.tile([P, D], F32, tag="ofin",
                                          name="ofin")
                            nc.vector.tensor_scalar_mul(
                                of[:], oacc[:, m * D:(m + 1) * D], rcp[:])
                            nc.sync.dma_start(out_p[m * P:(m + 1) * P, :],
                                              of[:])

                    def emit_step(tr, pv):
                        # Interleave the LDW-bound PE transposes of pending
                        # `tr` between the PV matmuls of pending `pv` so the
                        # transpose weight loads hide under the 213ns PV MMs.
                        tp = None
                        if tr is not None:
                            tp = tps.tile([P, CB], BF16, tag="tp", name="tp")
                        o_halves = None
                        if pv is not None:
                            o_halves = [ops.tile([P, CB], F32, tag="opart",
                                                 name="o_part")
                                        for _ in range(NH)]
                        for tc_ in range(TC):
                            if tr is not None:
                                nc.tensor.transpose(
                                    tp[:, tc_ * P:(tc_ + 1) * P],
                                    tr[0][:, tc_ * P:(tc_ + 1) * P],
                                    ident_bf[:],
                                )
                            if pv is not None:
                                pt, xb = pv[5], pv[4]
                                for h in range(NH):
                                    nc.tensor.matmul(
                                        o_halves[h][:],
                                        pt[:, tc_ * P:(tc_ + 1) * P],
                                        xb[:, tc_ * D + h * CB:
                                           tc_ * D + (h + 1) * CB],
                                        start=(tc_ == 0),
                                        stop=(tc_ == TC - 1),
                                    )
                        if tr is not None:
                            pt_new = ptp.tile([P, CB], BF16, tag="pt",
                                              name="pt")
                            nc.scalar.copy(pt_new[:], tp[:])
                            tr[5] = pt_new
                        if pv is not None:
                            flush_dve(pv, o_halves)

                    pend_s = []   # stats done, needs transpose
                    pend_t = []   # transposed, needs PV
                    for j in range(NV):
                        if j == 0:
                            kt, xb = kt0, xb0
                        elif j == 1:
                            kt, xb = kt1, xb1
                        else:
                            kt = load_kt(j, nc.sync if j % 2 == 0
                                         else nc.scalar,
                                         pid_sy if j % 2 == 0 else pid_sc)
                            xb = load_xb(j)

                        for m in range(MC):
                            sh = sps.tile([P, CB], F32, tag="s", name="s")
                            for k in range(DC):
                                nc.tensor.matmul(
                                    sh[:],
                                    tq[:, k * SL + m * P:
                                       k * SL + (m + 1) * P],
                                    kt[:, k * CB:(k + 1) * CB],
                                    start=(k == 0), stop=(k == DC - 1),
                                )
                            mq = stp.tile([P, 1], F32, tag="mq", name="mq")
                            nc.vector.reduce_max(mq[:], sh[:],
                                                 axis=mybir.AxisListType.X)

                            # online softmax stats; mst ping-pongs on j
                            m_old = mst[m][j % 2]
                            mnew = mst[m][(j + 1) % 2]
                            nc.vector.tensor_max(mnew[:], m_old[:], mq[:])
                            nbias = stp.tile([P, 1], F32, tag="nbias",
                                             name="nbias")
                            nc.scalar.mul(nbias[:], mnew[:], -SCALE)
                            # alpha = exp((m_old - mnew)/32)
                            alpha = stp.tile([P, 1], F32, tag="alpha",
                                             name="alpha")
                            nc.scalar.activation(alpha[:], m_old[:],
                                                 ACTF.Exp,
                                                 bias=nbias[:], scale=SCALE)

                            # phat = exp(s/32 - mnew/32) in bf16; sums in sq
                            sq = stp.tile([P, 1], F32, tag="sq", name="sq")
                            ph = php.tile([P, CB], BF16, tag="ph",
                                          name="ph")
                            nc.scalar.activation(ph[:], sh[:], ACTF.Exp,
                                                 bias=nbias[:], scale=SCALE,
                                                 accum_out=sq[:])
                            nc.vector.scalar_tensor_tensor(
                                sig[m][:], sig[m][:], alpha[:], sq[:],
                                op0=ALU.mult, op1=ALU.add)

                            pend_s.append([ph, alpha, m, j, xb, None])
                            if len(pend_s) >= 2:
                                tr = pend_s.pop(0)
                                pv = pend_t.pop(0) if pend_t else None
                                emit_step(tr, pv)
                                pend_t.append(tr)
                    while pend_s or pend_t:
                        tr = pend_s.pop(0) if pend_s else None
                        pv = pend_t.pop(0) if pend_t else None
                        emit_step(tr, pv)
                        if tr is not None:
                            pend_t.append(tr)

    nc.compile()
    return nc


_PROGRAM = None


def _get_program():
    global _PROGRAM
    if _PROGRAM is None:
        _PROGRAM = build_program()
    return _PROGRAM


def kernel(inputs, rotation_params, entangle_params, _trace=False):
    X = np.ascontiguousarray(np.asarray(inputs, dtype=np.float32))
    R = np.ascontiguousarray(np.asarray(rotation_params, dtype=np.float32))
    E = np.ascontiguousarray(np.asarray(entangle_params, dtype=np.float32))
    assert X.shape == (S, D) and R.shape == (D, D) and E.shape == (D, D)

    import ml_dtypes
    XT = np.ascontiguousarray(X.T)
    RT = np.ascontiguousarray(R.T)
    ET = np.ascontiguousarray(E.T)
    # X^T pre-blocked by key block in fp16: [NV, D, CB] -> [NV*D, CB]
    XTB = np.ascontiguousarray(
        XT.astype(np.float16).reshape(D, NV, CB)
        .transpose(1, 0, 2)).reshape(NV * D, CB)
    Xbf = np.ascontiguousarray(X.astype(ml_dtypes.bfloat16))
    in_maps = []
    for i in range(NCORES):
        in_maps.append({
            "rt": RT,
            "et": ET,
            "xtb": XTB,
            "xt": np.ascontiguousarray(XT[:, i * SL:(i + 1) * SL]),
            "xbf": Xbf,
        })

    nc = _get_program()
    res = run_bass_kernel_spmd(nc, in_maps, list(range(NCORES)),
                               trace=_trace)
    out = np.concatenate([res.results[i]["out"] for i in range(NCORES)],
                         axis=0)
    if _trace:
        return out, res
    return out


# revision 28
# speedup vs baseline: 1.3343x; 1.0308x over previous
"""Trainium2 Bass kernel for ClassicalSelfAttention.

  out = softmax((X @ R) @ (X @ E).T / sqrt(D)) @ X,  X: (8192, 1024) fp32

Key identity: scores = (X R)(X E)^T = X (R E^T) X^T.  Each core computes
W = R @ E^T redundantly (27us of PE), projects its own query slice
(T^T with lhsT=W, rhs=X^T_own), and then the "keys" are just X^T itself
— which every core already holds in DRAM.  No collectives at all: the
attention loop streams X^T / X blocks straight from HBM.

Sequence-parallel over 8 NeuronCores: core i owns queries
[i*1024, (i+1)*1024).  Attention runs over 16 key blocks of 512 in ring
order starting at the core's own two blocks, with a standard online
softmax (per-query running max on DVE, exp on ACT, PV accumulated in
per-half PSUM banks then merged into SBUF by DVE).  P^T for the PV
matmul is produced by PE transposes against a bf16 identity,
interleaved between the previous pending's PV matmuls so the
LDW-bound transposes hide under 213ns PV streams.

Precision: the softmax is extremely peaked (logit std ~1200 scaled;
argmax gaps down to ~3), so logits need >=11-bit operand mantissas —
bf16 flips argmaxes and fails.  QK runs in fp16 x fp16 (exact MACs,
one-pass LDWEIGHTS); W and T^T accumulate in fp32 with f32r / fp16
operands; P is cast to bf16 by the exp activation; PV runs bf16 with
fp32 PSUM accumulation.  Measured rel err vs the fp64 oracle: 1.54e-2
(threshold 2e-2) at ~565us, 84% MFU.

Startup DMAs are split by first-consumer order (fp16 rt in i-pair
slabs, fp16 et in halves, X^T_own halves last) across the two HWDGE
queues so the first W matmul issues at ~14us; xb blocks stream on the
gpsimd software-DGE queue.
"""
import numpy as np

import concourse.bass as bass_mod
import concourse.bacc as bacc
import concourse.mybir as mybir
from concourse import tile
from concourse.bass_utils import run_bass_kernel_spmd
from concourse.masks import make_identity

DT = mybir.dt
F32 = DT.float32
F32R = DT.float32r
BF16 = DT.bfloat16
F16 = DT.float16
ALU = mybir.AluOpType
ACTF = mybir.ActivationFunctionType

S, D, NCORES = 8192, 1024, 8
SL = S // NCORES          # 1024 queries per core
P = 128                   # partitions
DC = D // P               # 8 contraction chunks
MC = SL // P              # 8 query chunks per core
CB = 512                  # key block size
NV = S // CB              # 16 key blocks
TC = CB // P              # 4 t-chunks per block
SCALE = 1.0 / 32.0        # 1/sqrt(D)
NEG_BIG = -1.0e30


def build_program(num_devices=NCORES):
    nc = bacc.Bacc("TRN2", target_bir_lowering=False, debug=False,
                   num_devices=num_devices)

    rt_p = nc.declare_dram_parameter("rt", [D, D], F16, isOutput=False)
    et_p = nc.declare_dram_parameter("et", [D, D], F16, isOutput=False)
    # X^T pre-blocked by key block: [NV, D, CB] flattened to [NV*D, CB]
    xtb_p = nc.declare_dram_parameter("xtb", [NV * D, CB], F16,
                                      isOutput=False)
    # own X^T slice in f32r for the T^T projection (Q side stays clean)
    xt_p = nc.declare_dram_parameter("xt", [D, SL], F32R, isOutput=False)
    xbf_p = nc.declare_dram_parameter("xbf", [S, D], BF16, isOutput=False)
    out_p = nc.declare_dram_parameter("out", [SL, D], F32, isOutput=True)

    with tile.TileContext(nc) as tc:
        with (
            tc.tile_pool(name="persist", bufs=1) as pers,
        ):
            tq = pers.tile([P, DC * SL], F16, tag="tq")       # T^T, [c|m]
            oacc = pers.tile([P, MC * D], F32, tag="oacc")    # O accum per m
            ident_bf = pers.tile([P, P], BF16, tag="identbf")
            mst = [[pers.tile([P, 1], F32, tag=f"mst{m}_{j}",
                              name=f"mst{m}_{j}")
                    for j in range(2)] for m in range(MC)]
            sig = [pers.tile([P, 1], F32, tag=f"sig{m}", name=f"sig{m}")
                   for m in range(MC)]

            with tc.tile_pool(name="ident_tmp", bufs=1) as identp:
                ident32 = identp.tile([P, P], F32, tag="ident32")
                make_identity(nc, ident32[:])
                nc.vector.tensor_copy(ident_bf[:], ident32[:])
            nc.vector.memset(oacc[:], 0.0)
            for m in range(MC):
                nc.vector.memset(mst[m][0][:], NEG_BIG)
                nc.vector.memset(sig[m][:], 0.0)

            pid_sy = nc.sync.partition_id()
            pid_sc = nc.scalar.partition_id()
            pid_gp = nc.gpsimd.partition_id()

            with (
                tc.tile_pool(name="kt", bufs=2) as ktp,
                tc.tile_pool(name="xb", bufs=2) as xbp,
            ):
                def load_kt(j, eng, pid):
                    kt = ktp.tile([P, DC * CB], F16, tag="kt", name="kt")
                    eng.dma_start(
                        kt.rearrange("p (k c) -> p k c", k=DC),
                        xtb_p[bass_mod.ds(
                            ((pid * 2 + j) % NV) * D, D), :]
                        .rearrange("(k p) c -> p k c", p=P))
                    return kt

                def load_xb(j):
                    xb = xbp.tile([P, TC * D], BF16, tag="xb", name="xb")
                    nc.gpsimd.dma_start(
                        xb.rearrange("p (k c) -> p k c", k=TC),
                        xbf_p[bass_mod.ds(
                            ((pid_gp * 2 + j) % NV) * CB, CB), :]
                        .rearrange("(k p) c -> p k c", p=P))
                    return xb

                # own key blocks double as X^T_own for the T^T projection
                xb0 = load_xb(0)
                xb1 = load_xb(1)

                # ------------ Phase A: W = R @ E^T, then T^T ------------
                with (
                    tc.tile_pool(name="pa", bufs=1) as pa,
                    tc.tile_pool(name="rt", bufs=3) as rtp,
                    tc.tile_pool(name="pa_ps", bufs=2, space="PSUM") as pa_ps,
                ):
                    et_sb = pa.tile([P, DC * D], F16, tag="et")   # [d | j]
                    w_sb = pa.tile([P, DC * D], F32R, tag="w")    # [i | j]
                    xt_sb = pa.tile([P, DC * SL], F32R, tag="xt")  # [d | m]
                    et_r = et_sb.rearrange("p (k c) -> p k c", k=DC)
                    et_src = et_p.rearrange("(k p) c -> p k c", p=P)
                    for jh in range(2):
                        nc.scalar.dma_start(
                            et_r[:, :, jh * CB:(jh + 1) * CB],
                            et_src[:, :, jh * CB:(jh + 1) * CB])
                    xt_r = xt_sb.rearrange("p (k c) -> p k c", k=DC)
                    xt_src = xt_p.rearrange("(k p) c -> p k c", p=P)
                    nc.scalar.dma_start(xt_r[:, :, CB:2 * CB],
                                        xt_src[:, :, CB:2 * CB])
                    kt1 = load_kt(1, nc.scalar, pid_sc)
                    rt_src = rt_p.rearrange("(k p) c -> p k c", p=P)

                    def load_rt(g):
                        rt_t = rtp.tile([P, DC * 2 * P], F16, tag="rtg",
                                        name="rtg")
                        nc.sync.dma_start(
                            rt_t.rearrange("p (k c) -> p k c", k=DC),
                            rt_src[:, :, g * 2 * P:(g + 1) * 2 * P])
                        return rt_t

                    # W[i, j] = sum_d R^T[d, i] * E^T[d, j]
                    # rt streamed in i-pair slabs of [128, 8k x 256]
                    NG = DC // 2
                    rt_tiles = {0: load_rt(0), 1: load_rt(1)}
                    kt0 = None
                    for g in range(NG):
                        rt_t = rt_tiles.pop(g)
                        for i2 in range(2):
                            i = g * 2 + i2
                            for jh in range(2):
                                ps = pa_ps.tile([P, CB], F32, tag="proj")
                                for k in range(DC):
                                    nc.tensor.matmul(
                                        ps[:],
                                        rt_t[:, k * 2 * P + i2 * P:
                                             k * 2 * P + (i2 + 1) * P],
                                        et_sb[:, k * D + jh * CB:
                                              k * D + (jh + 1) * CB],
                                        start=(k == 0), stop=(k == DC - 1),
                                    )
                                nc.vector.tensor_copy(
                                    w_sb[:, i * D + jh * CB:
                                         i * D + (jh + 1) * CB],
                                    ps[:])
                        if g + 2 < NG:
                            rt_tiles[g + 2] = load_rt(g + 2)
                        elif g + 2 == NG:
                            # own X^T half + kt block queue after rt slabs
                            nc.sync.dma_start(xt_r[:, :, 0:CB],
                                              xt_src[:, :, 0:CB])
                            kt0 = load_kt(0, nc.sync, pid_sy)

                    # T^T[c, m] = sum_d W[d, c] * X^T[d, m]
                    # mh=1 first: its X^T half lands on the less-loaded queue
                    for mh in (1, 0):
                        for c in range(DC):
                            ps = pa_ps.tile([P, CB], F32, tag="proj")
                            for k in range(DC):
                                nc.tensor.matmul(
                                    ps[:],
                                    w_sb[:, k * D + c * P:
                                         k * D + (c + 1) * P],
                                    xt_sb[:, k * SL + mh * CB:
                                          k * SL + (mh + 1) * CB],
                                    start=(k == 0), stop=(k == DC - 1),
                                )
                            nc.vector.tensor_copy(
                                tq[:, c * SL + mh * CB:
                                   c * SL + (mh + 1) * CB],
                                ps[:])

                # ------------- Phase B: blocked attention ---------------
                # 16 key blocks of 512 in ring order starting at the own
                # blocks.  Software-pipelined: PE runs PV of a previous
                # block's m while DVE/ACT compute stats+exp of the current.
                self_attention_pools = (
                    tc.tile_pool(name="ph", bufs=4),
                    tc.tile_pool(name="pt", bufs=3),
                    tc.tile_pool(name="of", bufs=2),
                    tc.tile_pool(name="stats", bufs=6),
                    tc.tile_pool(name="s_ps", bufs=3, space="PSUM"),
                    tc.tile_pool(name="t_ps", bufs=2, space="PSUM"),
                    tc.tile_pool(name="o_ps", bufs=3, space="PSUM"),
                )
                with (
                    self_attention_pools[0] as php,
                    self_attention_pools[1] as ptp,
                    self_attention_pools[2] as ofp,
                    self_attention_pools[3] as stp,
                    self_attention_pools[4] as sps,
                    self_attention_pools[5] as tps,
                    self_attention_pools[6] as ops,
                ):
                    NH = D // CB  # PV output halves (separate PSUM banks)
                    def flush_dve(pend, o_halves):
                        ph, alpha, m, j, xb, pt = pend
                        for h, o_h in enumerate(o_halves):
                            nc.vector.scalar_tensor_tensor(
                                oacc[:, m * D + h * CB:
                                     m * D + (h + 1) * CB],
                                oacc[:, m * D + h * CB:
                                     m * D + (h + 1) * CB],
                                alpha[:], o_h[:],
                                op0=ALU.mult, op1=ALU.add)
                        if j == NV - 1:
                            # finalize this m: divide by softmax sum, store
                            rcp = stp.tile([P, 1], F32, tag="rcp",
                                           name="rcp")
                            nc.vector.reciprocal(rcp[:], sig[m][:])
                            of = ofp.tile([P, D], F32, tag="ofin",
                                          name="ofin")
                            nc.vector.tensor_scalar_mul(
                                of[:], oacc[:, m * D:(m + 1) * D], rcp[:])
                            nc.sync.dma_start(out_p[m * P:(m + 1) * P, :],
                                              of[:])

                    def emit_step(tr, pv):
                        # Interleave the LDW-bound PE transposes of pending
                        # `tr` between the PV matmuls of pending `pv` so the
                        # transpose weight loads hide under the 213ns PV MMs.
                        tp = None
                        if tr is not None:
                            tp = tps.tile([P, CB], BF16, tag="tp", name="tp")
                        o_halves = None
                        if pv is not None:
                            o_halves = [ops.tile([P, CB], F32, tag="opart",
                                                 name="o_part")
                                        for _ in range(NH)]
                        for tc_ in range(TC):
                            if tr is not None:
                                nc.tensor.transpose(
                                    tp[:, tc_ * P:(tc_ + 1) * P],
                                    tr[0][:, tc_ * P:(tc_ + 1) * P],
                                    ident_bf[:],
                                )
                            if pv is not None:
                                pt, xb = pv[5], pv[4]
                                for h in range(NH):
                                    nc.tensor.matmul(
                                        o_halves[h][:],
                                        pt[:, tc_ * P:(tc_ + 1) * P],
                                        xb[:, tc_ * D + h * CB:
                                           tc_ * D + (h + 1) * CB],
                                        start=(tc_ == 0),
                                        stop=(tc_ == TC - 1),
                                    )
                        if tr is not None:
                            pt_new = ptp.tile([P, CB], BF16, tag="pt",
                                              name="pt")
                            nc.scalar.copy(pt_new[:], tp[:])
                            tr[5] = pt_new
                        if pv is not None:
                            flush_dve(pv, o_halves)

                    pend_s = []   # stats done, needs transpose
                    pend_t = []   # transposed, needs PV
                    for j in range(NV):
                        if j == 0:
                            kt, xb = kt0, xb0
                        elif j == 1:
                            kt, xb = kt1, xb1
                        else:
                            kt = load_kt(j, nc.sync if j % 2 == 0
                                         else nc.scalar,
                                         pid_sy if j % 2 == 0 else pid_sc)
                            xb = load_xb(j)

                        for m in range(MC):
                            sh = sps.tile([P, CB], F32, tag="s", name="s")
                            for k in range(DC):
                                nc.tensor.matmul(
                                    sh[:],
                                    tq[:, k * SL + m * P:
                                       k * SL + (m + 1) * P],
                                    kt[:, k * CB:(k + 1) * CB],
                                    start=(k == 0), stop=(k == DC - 1),
                                )
                            mq = stp.tile([P, 1], F32, tag="mq", name="mq")
                            nc.vector.reduce_max(mq[:], sh[:],
                                                 axis=mybir.AxisListType.X)

                            # online softmax stats; mst ping-pongs on j
                            m_old = mst[m][j % 2]
                            mnew = mst[m][(j + 1) % 2]
                            nc.vector.tensor_max(mnew[:], m_old[:], mq[:])
                            nbias = stp.tile([P, 1], F32, tag="nbias",
                                             name="nbias")
                            nc.scalar.mul(nbias[:], mnew[:], -SCALE)
                            # alpha = exp((m_old - mnew)/32)
                            alpha = stp.tile([P, 1], F32, tag="alpha",
                                             name="alpha")
                            nc.scalar.activation(alpha[:], m_old[:],
                                                 ACTF.Exp,
                                                 bias=nbias[:], scale=SCALE)

                            # phat = exp(s/32 - mnew/32) in bf16; sums in sq
                            sq = stp.tile([P, 1], F32, tag="sq", name="sq")
                            ph = php.tile([P, CB], BF16, tag="ph",
                                          name="ph")
                            nc.scalar.activation(ph[:], sh[:], ACTF.Exp,
                                                 bias=nbias[:], scale=SCALE,
                                                 accum_out=sq[:])
                            nc.vector.scalar_tensor_tensor(
                                sig[m][:], sig[m][:], alpha[:], sq[:],
                                op0=ALU.mult, op1=ALU.add)

                            pend_s.append([ph, alpha, m, j, xb, None])
                            if len(pend_s) >= 2:
                                tr = pend_s.pop(0)
                                pv = pend_t.pop(0) if pend_t else None
                                emit_step(tr, pv)
                                pend_t.append(tr)
                    while pend_s or pend_t:
                        tr = pend_s.pop(0) if pend_s else None
                        pv = pend_t.pop(0) if pend_t else None
                        emit_step(tr, pv)
                        if tr is not None:
                            pend_t.append(tr)

    nc.compile()
    return nc


_PROGRAM = None


def _get_program():
    global _PROGRAM
    if _PROGRAM is None:
        _PROGRAM = build_program()
    return _PROGRAM


def kernel(inputs, rotation_params, entangle_params, _trace=False):
    X = np.ascontiguousarray(np.asarray(inputs, dtype=np.float32))
    R = np.ascontiguousarray(np.asarray(rotation_params, dtype=np.float32))
    E = np.ascontiguousarray(np.asarray(entangle_params, dtype=np.float32))
    assert X.shape == (S, D) and R.shape == (D, D) and E.shape == (D, D)

    import ml_dtypes
    XT = np.ascontiguousarray(X.T)
    RT = np.ascontiguousarray(R.T.astype(np.float16))
    ET = np.ascontiguousarray(E.T.astype(np.float16))
    # X^T pre-blocked by key block in fp16: [NV, D, CB] -> [NV*D, CB]
    XTB = np.ascontiguousarray(
        XT.astype(np.float16).reshape(D, NV, CB)
        .transpose(1, 0, 2)).reshape(NV * D, CB)
    Xbf = np.ascontiguousarray(X.astype(ml_dtypes.bfloat16))
    in_maps = []
    for i in range(NCORES):
        in_maps.append({
            "rt": RT,
            "et": ET,
            "xtb": XTB,
            "xt": np.ascontiguousarray(XT[:, i * SL:(i + 1) * SL]),
            "xbf": Xbf,
        })

    nc = _get_program()
    res = run_bass_kernel_spmd(nc, in_maps, list(range(NCORES)),
                               trace=_trace)
    out = np.concatenate([res.results[i]["out"] for i in range(NCORES)],
                         axis=0)
    if _trace:
        return out, res
    return out
